# revision 7
# baseline (speedup 1.0000x reference)
"""BipartiteGConv Trainium2 kernel (8 NeuronCores, edge/node sharding).

Math (see reference):
  rhs = input @ Wi + bi            [N_IN, D]
  lhs = other @ Wo                 [N_OT, D]
  msg = lrelu(rhs[rj] + lhs[lj] + w*We) per edge
  S   = segment_sum(msg, rj)       [N_IN, D]
  out = concat([S @ Wf + bf, input]) @ Wout + bout
      = S @ (Wf@W1) + counts x (bf@W1) + input @ W2 + bout   (W1=Wout[:D], W2=Wout[D:])

Sharding: nodes (rj ranges of 12500) across 8 cores; each core owns all
edges targeting its range.  Slots ordered by (lj-segment, rj-window of 128
nodes), padded per (window, seg) to uniform tile counts across cores
(SPMD).  Gathers via dma_gather (bf16-padded 256B rows); segment-sum via
per-tile onehot matmul accumulated in PSUM per window.
"""
import sys
sys.path.insert(0, "/opt/trn_rl_repo")
import numpy as np
import ml_dtypes

N_IN, N_OT, E, D = 100000, 50000, 1000000, 64
NC = 8
NPC = N_IN // NC            # nodes per core
W = 128                     # window size (nodes)
NW = (NPC + W - 1) // W     # windows per core
SEG_SPLIT = 32768           # lhs table A/B split (int16 idx limit)
BLK = 1024                  # gather tokens per instruction
PADV = 999.0                # rj_local value for pad slots (onehot -> 0)


def _wrap16(a):
    # token i -> [i % 16, i // 16], replicated to 128 partitions
    n = a.shape[0]
    assert n % 16 == 0
    return np.tile(a.reshape(n // 16, 16).T, (8, 1)).copy()


def kernel(input, other, rj, lj, weights, Wi, bi, Wo, We, Wf, bf, Wout, bout):
    import concourse.bass as bass
    import concourse.bacc as bacc
    import concourse.mybir as mybir
    import concourse.tile as tile
    from concourse.bass_utils import run_bass_kernel_spmd
    from contextlib import ExitStack

    input = np.asarray(input, np.float32)
    other = np.asarray(other, np.float32)
    rj = np.asarray(rj).astype(np.int64)
    lj = np.asarray(lj).astype(np.int64)
    weights = np.asarray(weights, np.float32).reshape(-1)
    Wi = np.asarray(Wi, np.float32); bi = np.asarray(bi, np.float32)
    Wo = np.asarray(Wo, np.float32); We = np.asarray(We, np.float32).reshape(-1)
    Wf = np.asarray(Wf, np.float32); bf = np.asarray(bf, np.float32)
    Wout = np.asarray(Wout, np.float32); bout = np.asarray(bout, np.float32)

    bf16 = ml_dtypes.bfloat16

    # ---------------- host index prep (per core) ----------------
    core_of = rj // NPC
    order0 = np.argsort(core_of, kind="stable")
    # per (core, seg, window) edge lists
    tiles_per = np.zeros((NC, 2, NW), np.int64)
    core_data = []
    for c in range(NC):
        sel = order0[np.searchsorted(core_of, c, side="left", sorter=order0):
                     np.searchsorted(core_of, c, side="right", sorter=order0)]
        sel = order0[core_of[order0] == c] if False else sel
        rjl_all = rj[sel] - c * NPC
        win = rjl_all // W
        seg = (lj[sel] >= SEG_SPLIT).astype(np.int64)
        key = seg * NW + win
        o2 = np.argsort(key, kind="stable")
        core_data.append((sel[o2], (rjl_all % W)[o2], key[o2]))
        cnt = np.bincount(key[o2], minlength=2 * NW).reshape(2, NW)
        tiles_per[c] = (cnt + 127) // 128
    TW = tiles_per.max(axis=0)          # uniform tiles per (seg, window)
    # segment A tile count rounded so segment boundary is BLK-aligned
    TA = int(TW[0].sum()); TB = int(TW[1].sum())
    padA = (-TA) % (BLK // 128)
    padB = (-(TA + padA + TB)) % (BLK // 128)
    # tile schedule: list of (seg, window) per tile, with pad tiles (seg, -1)
    sched = []
    for w in range(NW):
        sched += [(0, w)] * int(TW[0, w])
    sched += [(0, -1)] * padA
    for w in range(NW):
        sched += [(1, w)] * int(TW[1, w])
    sched += [(1, -1)] * padB
    T = len(sched)
    S = T * 128
    TA_tok = (TA + padA) * 128          # segment A token count (BLK-aligned)

    rhs_idx = np.zeros((NC, S), np.int16)
    lhs_idx = np.zeros((NC, S), np.int16)
    rjl_grid = np.full((NC, S), PADV, np.float32)
    w_grid = np.zeros((NC, S), np.float32)
    counts = np.zeros((NC, NPC), np.float32)
    for c in range(NC):
        sel, rjl_loc, key = core_data[c]
        counts[c] = np.bincount(rj[sel] - c * NPC, minlength=NPC)
        # slot positions: walk schedule, fill each (seg,window) group
        pos = 0
        ptr = 0  # pointer into sel
        for (sg, w) in sched:
            if w < 0:
                pos += 128
                continue
            k = sg * NW + w
            # edges for this (seg, window) in this core
            lo = np.searchsorted(key, k, side="left")
            hi = np.searchsorted(key, k, side="right")
            ne = hi - lo
            ntile = int(TW[sg, w])
            # how many already consumed for this key from earlier tiles
            # (groups are contiguous; fill greedily tile by tile)
            take = min(128, ne - (ptr if False else 0))
            # simpler: fill the whole group at its first tile encounter
            if lo != hi:
                e0 = np.arange(lo, hi)
                p0 = pos  # this is the first tile of the group only if we track it
            pos += 128
        # vectorized fill instead (group-contiguous):
        pos_of_group = {}
        p = 0
        for (sg, w) in sched:
            if w >= 0 and (sg, w) not in pos_of_group:
                pos_of_group[(sg, w)] = p
            p += 128
        for sg in range(2):
            for w in range(NW):
                k = sg * NW + w
                lo = np.searchsorted(key, k, side="left")
                hi = np.searchsorted(key, k, side="right")
                if lo == hi:
                    continue
                base = pos_of_group[(sg, w)]
                idxs = np.arange(base, base + (hi - lo))
                ee = sel[lo:hi]
                rhs_idx[c, idxs] = (rj[ee] - c * NPC).astype(np.int16)
                lv = lj[ee] - sg * SEG_SPLIT
                lhs_idx[c, idxs] = lv.astype(np.int16)
                rjl_grid[c, idxs] = rjl_loc[lo:hi].astype(np.float32)
                w_grid[c, idxs] = weights[ee]

    # slot s maps to (p, t) = (s % 128, s // 128) [dma_gather token layout]
    def grid_pt(a, dt):
        return np.ascontiguousarray(a.reshape(T, 128).T).astype(dt)

    # ---------------- build bass kernel ----------------
    dt = mybir.dt
    nc = bacc.Bacc("TRN2", target_bir_lowering=False, debug=False,
                   num_devices=NC, num_swdge_queues=4)

    inT_ext = nc.dram_tensor("inT", [65, NPC], dt.bfloat16, kind="ExternalInput").ap()
    otT_ext = nc.dram_tensor("otT", [64, N_OT], dt.bfloat16, kind="ExternalInput").ap()
    WiB_ext = nc.dram_tensor("WiB", [65, 64], dt.bfloat16, kind="ExternalInput").ap()
    Wo_ext = nc.dram_tensor("Wo_", [64, 64], dt.bfloat16, kind="ExternalInput").ap()
    M1_ext = nc.dram_tensor("M1_", [64, 64], dt.bfloat16, kind="ExternalInput").ap()
    W2_ext = nc.dram_tensor("W2_", [64, 64], dt.bfloat16, kind="ExternalInput").ap()
    vb_ext = nc.dram_tensor("vb_", [2, 64], dt.bfloat16, kind="ExternalInput").ap()
    cnts_ext = nc.dram_tensor("cnts", [2, NPC], dt.bfloat16, kind="ExternalInput").ap()
    WeR_ext = nc.dram_tensor("WeR", [128, 64], dt.bfloat16, kind="ExternalInput").ap()
    iota_ext = nc.dram_tensor("iot", [128, 128], dt.bfloat16, kind="ExternalInput").ap()
    rix_ext = nc.dram_tensor("rix", [128, S // 16], dt.int16, kind="ExternalInput").ap()
    lix_ext = nc.dram_tensor("lix", [128, S // 16], dt.int16, kind="ExternalInput").ap()
    rjl_ext = nc.dram_tensor("rjl", [128, T], dt.float32, kind="ExternalInput").ap()
    wg_ext = nc.dram_tensor("wg", [128, T], dt.float32, kind="ExternalInput").ap()
    y_ext = nc.dram_tensor("y", [NPC, 64], dt.float32, kind="ExternalOutput").ap()

    rtab = nc.dram_tensor("rtab", [NPC, 128], dt.bfloat16).ap()
    ltab = nc.dram_tensor("ltab", [N_OT, 128], dt.bfloat16).ap()

    NBLK = S // BLK
    TPB = BLK // 128  # tiles per block = 8

    with tile.TileContext(nc) as tc, ExitStack() as ctx:
        cpool = ctx.enter_context(tc.tile_pool(name="const", bufs=1))
        tabp = ctx.enter_context(tc.tile_pool(name="tab", bufs=3))
        gp = ctx.enter_context(tc.tile_pool(name="gath", bufs=6))
        wk = ctx.enter_context(tc.tile_pool(name="work", bufs=4))
        ohp = ctx.enter_context(tc.tile_pool(name="ohp", bufs=2 * TPB + 2))
        psA = ctx.enter_context(tc.tile_pool(name="psA", bufs=2, space="PSUM"))
        psW = ctx.enter_context(tc.tile_pool(name="psW", bufs=2, space="PSUM"))
        accp = ctx.enter_context(tc.tile_pool(name="acc", bufs=1))

        iota = cpool.tile([128, 128], dt.bfloat16)
        nc.sync.dma_start(out=iota[:], in_=iota_ext[:])
        WeR = cpool.tile([128, 64], dt.bfloat16)
        nc.sync.dma_start(out=WeR[:], in_=WeR_ext[:])
        WiB = cpool.tile([65, 64], dt.bfloat16)
        nc.sync.dma_start(out=WiB[:], in_=WiB_ext[:])
        Wo_t = cpool.tile([64, 64], dt.bfloat16)
        nc.sync.dma_start(out=Wo_t[:], in_=Wo_ext[:])
        rjl = cpool.tile([128, T], dt.float32)
        nc.sync.dma_start(out=rjl[:], in_=rjl_ext[:])
        wg = cpool.tile([128, T], dt.float32)
        nc.sync.dma_start(out=wg[:], in_=wg_ext[:])
        rix = cpool.tile([128, S // 16], dt.int16)
        nc.sync.dma_start(out=rix[:], in_=rix_ext[:])
        lix = cpool.tile([128, S // 16], dt.int16)
        nc.sync.dma_start(out=lix[:], in_=lix_ext[:])

        acc = accp.tile([128, NW, 64], dt.float32)
        nc.vector.memset(acc[:], 0.0)

        # ---- build rhs table [NPC,128] (cols 0:64 = input@Wi+bi, bf16) ----
        for w in range(NW):
            n0 = w * W
            n1 = min(NPC, n0 + W)
            m = n1 - n0
            aT = tabp.tile([65, 128], dt.bfloat16, tag="aT")
            nc.sync.dma_start(out=aT[:, :m], in_=inT_ext[:, n0:n1])
            ps = psA.tile([128, 64], dt.float32, tag="tps")
            nc.tensor.matmul(out=ps[:m, :], lhsT=aT[:, :m], rhs=WiB[:], start=True, stop=True)
            sb = tabp.tile([128, 64], dt.bfloat16, tag="tsb")
            nc.vector.tensor_copy(out=sb[:m, :], in_=ps[:m, :])
            nc.sync.dma_start(out=rtab[n0:n1, 0:64], in_=sb[:m, :])
        # ---- build lhs table [N_OT,128] ----
        NWO = (N_OT + W - 1) // W
        for w in range(NWO):
            n0 = w * W
            n1 = min(N_OT, n0 + W)
            m = n1 - n0
            aT = tabp.tile([64, 128], dt.bfloat16, tag="bT")
            nc.sync.dma_start(out=aT[:, :m], in_=otT_ext[:, n0:n1])
            ps = psA.tile([128, 64], dt.float32, tag="tps")
            nc.tensor.matmul(out=ps[:m, :], lhsT=aT[:, :m], rhs=Wo_t[:], start=True, stop=True)
            sb = tabp.tile([128, 64], dt.bfloat16, tag="tsb")
            nc.vector.tensor_copy(out=sb[:m, :], in_=ps[:m, :])
            nc.sync.dma_start(out=ltab[n0:n1, 0:64], in_=sb[:m, :])

        # ---- main loop: gather blocks, per-tile message + window scatter ----
        # track PSUM window accumulation: flush when window changes
        cur = {"w": None, "seg": None, "ps": None, "n0": None}

        def flush():
            if cur["ps"] is not None:
                wv = cur["w"]
                nc.vector.tensor_tensor(out=acc[:, wv, :], in0=acc[:, wv, :],
                                        in1=cur["ps"][:], op=mybir.AluOpType.add)
                cur["ps"] = None

        t_global = 0
        for b in range(NBLK):
            segb = 0 if (b * BLK) < TA_tok else 1
            tab_ap = ltab[0:SEG_SPLIT, :] if segb == 0 else ltab[SEG_SPLIT:N_OT, :]
            gl = gp.tile([128, TPB, 128], dt.bfloat16, tag="gl")
            nc.gpsimd.dma_gather(gl[:], tab_ap, lix[:, b * (BLK // 16):(b + 1) * (BLK // 16)],
                                 BLK, BLK, 128, queue_num=b % 4)
            gr = gp.tile([128, TPB, 128], dt.bfloat16, tag="gr")
            nc.gpsimd.dma_gather(gr[:], rtab[:], rix[:, b * (BLK // 16):(b + 1) * (BLK // 16)],
                                 BLK, BLK, 128, queue_num=(b + 2) % 4)
            msum = wk.tile([128, TPB, 64], dt.bfloat16, tag="msum")
            nc.vector.tensor_tensor(out=msum[:], in0=gl[:, :, 0:64], in1=gr[:, :, 0:64],
                                    op=mybir.AluOpType.add)
            m2b = wk.tile([128, TPB, 64], dt.bfloat16, tag="m2b")
            ohs = []
            for i in range(TPB):
                t = t_global + i
                oh = ohp.tile([128, 128], dt.bfloat16, tag="oh", name=f"oh{t}")
                nc.vector.tensor_scalar(out=oh[:], in0=iota[:], scalar1=rjl[:, t:t + 1],
                                        scalar2=None, op0=mybir.AluOpType.is_equal)
                ohs.append(oh)
                wwe = wk.tile([128, 64], dt.bfloat16, tag="wwe")
                nc.vector.tensor_scalar(out=wwe[:], in0=WeR[:], scalar1=wg[:, t:t + 1],
                                        scalar2=None, op0=mybir.AluOpType.mult)
                nc.vector.tensor_tensor(out=m2b[:, i, :], in0=msum[:, i, :], in1=wwe[:],
                                        op=mybir.AluOpType.add)
            mrb = wk.tile([128, TPB, 64], dt.bfloat16, tag="mrb")
            nc.scalar.activation(out=mrb[:], in_=m2b[:],
                                 func=mybir.ActivationFunctionType.Lrelu, alpha=0.01)
            for i in range(TPB):
                t = t_global
                t_global += 1
                sg, wv = sched[t]
                if wv < 0:
                    continue
                if cur["w"] != wv or cur["seg"] != sg:
                    flush()
                    cur["w"], cur["seg"] = wv, sg
                    cur["ps"] = psW.tile([128, 64], dt.float32, tag="psw", name=f"psw{t}")
                    cur["first"] = True
                nc.tensor.matmul(out=cur["ps"][:], lhsT=ohs[i][:], rhs=mrb[:, i, :],
                                 start=cur["first"], stop=False)
                cur["first"] = False
        flush()

        # ---- final: y = S@M1 + counts x v1 + 1 x bout + input@W2 ----
        M1t = cpool.tile([64, 64], dt.bfloat16)
        nc.sync.dma_start(out=M1t[:], in_=M1_ext[:])
        W2t = cpool.tile([64, 64], dt.bfloat16)
        nc.sync.dma_start(out=W2t[:], in_=W2_ext[:])
        vbt = cpool.tile([2, 64], dt.bfloat16)
        nc.sync.dma_start(out=vbt[:], in_=vb_ext[:])
        cntr = cpool.tile([2, NPC], dt.bfloat16)
        nc.sync.dma_start(out=cntr[:], in_=cnts_ext[:])
        ident = cpool.tile([128, 128], dt.bfloat16)
        from concourse.masks import make_identity
        make_identity(nc, ident[:])

        for w in range(NW):
            n0 = w * W
            n1 = min(NPC, n0 + W)
            m = n1 - n0
            # S_w bf16 and its transpose
            swb = wk.tile([128, 64], dt.bfloat16, tag="swb")
            nc.vector.tensor_copy(out=swb[:], in_=acc[:, w, :])
            tps = psA.tile([128, 128], dt.bfloat16, tag="fps")
            nc.tensor.transpose(out=tps[0:64, :], in_=swb[:], identity=ident[:])
            swT = wk.tile([64, 128], dt.bfloat16, tag="swT")
            nc.vector.tensor_copy(out=swT[:], in_=tps[0:64, :])
            inw = wk.tile([65, 128], dt.bfloat16, tag="inw")
            nc.sync.dma_start(out=inw[:, :m], in_=inT_ext[:, n0:n1])
            ops = psA.tile([128, 64], dt.float32, tag="ops")
            nc.tensor.matmul(out=ops[:m, :], lhsT=swT[:, :m], rhs=M1t[:], start=True, stop=False)
            nc.tensor.matmul(out=ops[:m, :], lhsT=inw[0:64, :m], rhs=W2t[:], start=False, stop=False)
            nc.tensor.matmul(out=ops[:m, :], lhsT=cntr[:, n0:n1], rhs=vbt[:], start=False, stop=True)
            ob = wk.tile([128, 64], dt.float32, tag="ob")
            nc.vector.tensor_copy(out=ob[:m, :], in_=ops[:m, :])
            nc.sync.dma_start(out=y_ext[n0:n1, :], in_=ob[:m, :])

    nc.compile()

    # ---------------- host-side in_maps ----------------
    W1 = Wout[:64]; W2 = Wout[64:]
    M1 = (Wf @ W1).astype(np.float32)
    v1 = (bf @ W1).astype(np.float32)
    vb = np.stack([v1, bout]).astype(bf16)
    iota_np = np.tile(np.arange(128, dtype=np.float32)[None, :], (128, 1)).astype(bf16)
    WeR_np = np.tile(We[None, :], (128, 1)).astype(bf16)
    WiB_np = np.concatenate([Wi, bi[None, :]], 0).astype(bf16)

    in_maps = []
    for c in range(NC):
        sl = input[c * NPC:(c + 1) * NPC]
        inT = np.concatenate([sl.T, np.ones((1, NPC), np.float32)], 0).astype(bf16)
        in_maps.append({
            "inT": np.ascontiguousarray(inT),
            "otT": np.ascontiguousarray(other.T).astype(bf16),
            "WiB": WiB_np, "Wo_": Wo.astype(bf16),
            "M1_": M1.astype(bf16), "W2_": W2.astype(bf16), "vb_": vb,
            "cnts": np.stack([counts[c], np.ones(NPC, np.float32)]).astype(bf16),
            "WeR": WeR_np, "iot": iota_np,
            "rix": _wrap16(rhs_idx[c]), "lix": _wrap16(lhs_idx[c]),
            "rjl": grid_pt(rjl_grid[c], np.float32),
            "wg": grid_pt(w_grid[c], np.float32),
        })

    import os
    res = run_bass_kernel_spmd(nc, in_maps, list(range(NC)),
                               trace=bool(os.environ.get("KTRACE")))
    if os.environ.get("KTRACE") and res.exec_time_ns:
        print(f"HW exec time: {res.exec_time_ns} ns")
    out = np.concatenate([res.results[c]["y"] for c in range(NC)], 0)
    return out.astype(np.float32)



# revision 9
# speedup vs baseline: 1.0832x; 1.0832x over previous
"""BipartiteGConv Trainium2 kernel (8 NeuronCores, edge/node sharding).

Math (see reference):
  rhs = input @ Wi + bi            [N_IN, D]
  lhs = other @ Wo                 [N_OT, D]
  msg = lrelu(rhs[rj] + lhs[lj] + w*We) per edge
  S   = segment_sum(msg, rj)       [N_IN, D]
  out = concat([S @ Wf + bf, input]) @ Wout + bout
      = S @ (Wf@W1) + counts x (bf@W1) + input @ W2 + bout   (W1=Wout[:D], W2=Wout[D:])

Sharding: nodes (rj ranges of 12500) across 8 cores; each core owns all
edges targeting its range.  Slots ordered by (lj-segment, rj-window of 128
nodes), padded per (window, seg) to uniform tile counts across cores
(SPMD).  Gathers via dma_gather (bf16-padded 256B rows); segment-sum via
per-tile onehot matmul accumulated in PSUM per window.
"""
import sys
sys.path.insert(0, "/opt/trn_rl_repo")
import numpy as np
import ml_dtypes

N_IN, N_OT, E, D = 100000, 50000, 1000000, 64
NC = 8
NPC = N_IN // NC            # nodes per core
W = 128                     # window size (nodes)
NW = (NPC + W - 1) // W     # windows per core
SEG_SPLIT = 32768           # lhs table A/B split (int16 idx limit)
BLK = 1024                  # gather tokens per instruction
PADV = 999.0                # rj_local value for pad slots (onehot -> 0)


def _wrap16(a):
    # token i -> [i % 16, i // 16], replicated to 128 partitions
    n = a.shape[0]
    assert n % 16 == 0
    return np.tile(a.reshape(n // 16, 16).T, (8, 1)).copy()


def kernel(input, other, rj, lj, weights, Wi, bi, Wo, We, Wf, bf, Wout, bout):
    import concourse.bass as bass
    import concourse.bacc as bacc
    import concourse.mybir as mybir
    import concourse.tile as tile
    from concourse.bass_utils import run_bass_kernel_spmd
    from contextlib import ExitStack

    input = np.asarray(input, np.float32)
    other = np.asarray(other, np.float32)
    rj = np.asarray(rj).astype(np.int64)
    lj = np.asarray(lj).astype(np.int64)
    weights = np.asarray(weights, np.float32).reshape(-1)
    Wi = np.asarray(Wi, np.float32); bi = np.asarray(bi, np.float32)
    Wo = np.asarray(Wo, np.float32); We = np.asarray(We, np.float32).reshape(-1)
    Wf = np.asarray(Wf, np.float32); bf = np.asarray(bf, np.float32)
    Wout = np.asarray(Wout, np.float32); bout = np.asarray(bout, np.float32)

    bf16 = ml_dtypes.bfloat16

    # ---------------- host index prep (per core) ----------------
    core_of = rj // NPC
    order0 = np.argsort(core_of, kind="stable")
    # per (core, seg, window) edge lists
    tiles_per = np.zeros((NC, 2, NW), np.int64)
    core_data = []
    for c in range(NC):
        sel = order0[np.searchsorted(core_of, c, side="left", sorter=order0):
                     np.searchsorted(core_of, c, side="right", sorter=order0)]
        sel = order0[core_of[order0] == c] if False else sel
        rjl_all = rj[sel] - c * NPC
        win = rjl_all // W
        seg = (lj[sel] >= SEG_SPLIT).astype(np.int64)
        key = seg * NW + win
        o2 = np.argsort(key, kind="stable")
        core_data.append((sel[o2], (rjl_all % W)[o2], key[o2]))
        cnt = np.bincount(key[o2], minlength=2 * NW).reshape(2, NW)
        tiles_per[c] = (cnt + 127) // 128
    TW = tiles_per.max(axis=0)          # uniform tiles per (seg, window)
    # segment A tile count rounded so segment boundary is BLK-aligned
    TA = int(TW[0].sum()); TB = int(TW[1].sum())
    padA = (-TA) % (BLK // 128)
    padB = (-(TA + padA + TB)) % (BLK // 128)
    # tile schedule: list of (seg, window) per tile, with pad tiles (seg, -1)
    sched = []
    for w in range(NW):
        sched += [(0, w)] * int(TW[0, w])
    sched += [(0, -1)] * padA
    for w in range(NW):
        sched += [(1, w)] * int(TW[1, w])
    sched += [(1, -1)] * padB
    T = len(sched)
    S = T * 128
    TA_tok = (TA + padA) * 128          # segment A token count (BLK-aligned)

    rhs_idx = np.zeros((NC, S), np.int16)
    lhs_idx = np.zeros((NC, S), np.int16)
    rjl_grid = np.full((NC, S), PADV, np.float32)
    w_grid = np.zeros((NC, S), np.float32)
    counts = np.zeros((NC, NPC), np.float32)
    for c in range(NC):
        sel, rjl_loc, key = core_data[c]
        counts[c] = np.bincount(rj[sel] - c * NPC, minlength=NPC)
        # slot positions: walk schedule, fill each (seg,window) group
        pos = 0
        ptr = 0  # pointer into sel
        for (sg, w) in sched:
            if w < 0:
                pos += 128
                continue
            k = sg * NW + w
            # edges for this (seg, window) in this core
            lo = np.searchsorted(key, k, side="left")
            hi = np.searchsorted(key, k, side="right")
            ne = hi - lo
            ntile = int(TW[sg, w])
            # how many already consumed for this key from earlier tiles
            # (groups are contiguous; fill greedily tile by tile)
            take = min(128, ne - (ptr if False else 0))
            # simpler: fill the whole group at its first tile encounter
            if lo != hi:
                e0 = np.arange(lo, hi)
                p0 = pos  # this is the first tile of the group only if we track it
            pos += 128
        # vectorized fill instead (group-contiguous):
        pos_of_group = {}
        p = 0
        for (sg, w) in sched:
            if w >= 0 and (sg, w) not in pos_of_group:
                pos_of_group[(sg, w)] = p
            p += 128
        for sg in range(2):
            for w in range(NW):
                k = sg * NW + w
                lo = np.searchsorted(key, k, side="left")
                hi = np.searchsorted(key, k, side="right")
                if lo == hi:
                    continue
                base = pos_of_group[(sg, w)]
                idxs = np.arange(base, base + (hi - lo))
                ee = sel[lo:hi]
                rhs_idx[c, idxs] = (rj[ee] - c * NPC).astype(np.int16)
                lv = lj[ee] - sg * SEG_SPLIT
                lhs_idx[c, idxs] = lv.astype(np.int16)
                rjl_grid[c, idxs] = rjl_loc[lo:hi].astype(np.float32)
                w_grid[c, idxs] = weights[ee]

    # slot s maps to (p, t) = (s % 128, s // 128) [dma_gather token layout]
    def grid_pt(a, dt):
        return np.ascontiguousarray(a.reshape(T, 128).T).astype(dt)

    # ---------------- build bass kernel ----------------
    dt = mybir.dt
    nc = bacc.Bacc("TRN2", target_bir_lowering=False, debug=False,
                   num_devices=NC, num_swdge_queues=4)

    inT_ext = nc.dram_tensor("inT", [65, NPC], dt.bfloat16, kind="ExternalInput").ap()
    otT_ext = nc.dram_tensor("otT", [64, N_OT], dt.bfloat16, kind="ExternalInput").ap()
    WiB_ext = nc.dram_tensor("WiB", [65, 64], dt.bfloat16, kind="ExternalInput").ap()
    Wo_ext = nc.dram_tensor("Wo_", [64, 64], dt.bfloat16, kind="ExternalInput").ap()
    M1_ext = nc.dram_tensor("M1_", [64, 64], dt.bfloat16, kind="ExternalInput").ap()
    W2_ext = nc.dram_tensor("W2_", [64, 64], dt.bfloat16, kind="ExternalInput").ap()
    vb_ext = nc.dram_tensor("vb_", [2, 64], dt.bfloat16, kind="ExternalInput").ap()
    cnts_ext = nc.dram_tensor("cnts", [2, NPC], dt.bfloat16, kind="ExternalInput").ap()
    WeR_ext = nc.dram_tensor("WeR", [128, 64], dt.bfloat16, kind="ExternalInput").ap()
    iota_ext = nc.dram_tensor("iot", [128, 128], dt.bfloat16, kind="ExternalInput").ap()
    rix_ext = nc.dram_tensor("rix", [128, S // 16], dt.int16, kind="ExternalInput").ap()
    lix_ext = nc.dram_tensor("lix", [128, S // 16], dt.int16, kind="ExternalInput").ap()
    rjl_ext = nc.dram_tensor("rjl", [128, T], dt.float32, kind="ExternalInput").ap()
    wg_ext = nc.dram_tensor("wg", [128, T], dt.float32, kind="ExternalInput").ap()
    y_ext = nc.dram_tensor("y", [NPC, 64], dt.float32, kind="ExternalOutput").ap()

    rtab = nc.dram_tensor("rtab", [NPC, 128], dt.bfloat16).ap()
    ltab = nc.dram_tensor("ltab", [N_OT, 128], dt.bfloat16).ap()

    NBLK = S // BLK
    TPB = BLK // 128  # tiles per block = 8

    with tile.TileContext(nc) as tc, ExitStack() as ctx:
        cpool = ctx.enter_context(tc.tile_pool(name="const", bufs=1))
        tabp = ctx.enter_context(tc.tile_pool(name="tab", bufs=3))
        gp = ctx.enter_context(tc.tile_pool(name="gath", bufs=6))
        wk = ctx.enter_context(tc.tile_pool(name="work", bufs=4))
        ohp = ctx.enter_context(tc.tile_pool(name="ohp", bufs=2 * TPB + 2))
        psA = ctx.enter_context(tc.tile_pool(name="psA", bufs=2, space="PSUM"))
        psW = ctx.enter_context(tc.tile_pool(name="psW", bufs=2, space="PSUM"))
        accp = ctx.enter_context(tc.tile_pool(name="acc", bufs=1))

        iota = cpool.tile([128, 128], dt.bfloat16)
        nc.sync.dma_start(out=iota[:], in_=iota_ext[:])
        WeR = cpool.tile([128, 64], dt.bfloat16)
        nc.sync.dma_start(out=WeR[:], in_=WeR_ext[:])
        WiB = cpool.tile([65, 64], dt.bfloat16)
        nc.sync.dma_start(out=WiB[:], in_=WiB_ext[:])
        Wo_t = cpool.tile([64, 64], dt.bfloat16)
        nc.sync.dma_start(out=Wo_t[:], in_=Wo_ext[:])
        rjl = cpool.tile([128, T], dt.float32)
        nc.sync.dma_start(out=rjl[:], in_=rjl_ext[:])
        wg = cpool.tile([128, T], dt.float32)
        nc.sync.dma_start(out=wg[:], in_=wg_ext[:])
        rix = cpool.tile([128, S // 16], dt.int16)
        nc.sync.dma_start(out=rix[:], in_=rix_ext[:])
        lix = cpool.tile([128, S // 16], dt.int16)
        nc.sync.dma_start(out=lix[:], in_=lix_ext[:])

        acc = accp.tile([128, NW, 64], dt.float32)
        nc.vector.memset(acc[:], 0.0)

        # ---- build rhs table [NPC,128] (cols 0:64 = input@Wi+bi, bf16) ----
        for w in range(NW):
            n0 = w * W
            n1 = min(NPC, n0 + W)
            m = n1 - n0
            aT = tabp.tile([65, 128], dt.bfloat16, tag="aT")
            nc.sync.dma_start(out=aT[:, :m], in_=inT_ext[:, n0:n1])
            ps = psA.tile([128, 64], dt.float32, tag="tps")
            nc.tensor.matmul(out=ps[:m, :], lhsT=aT[:, :m], rhs=WiB[:], start=True, stop=True)
            sb = tabp.tile([128, 64], dt.bfloat16, tag="tsb")
            nc.vector.tensor_copy(out=sb[:m, :], in_=ps[:m, :])
            nc.sync.dma_start(out=rtab[n0:n1, 0:64], in_=sb[:m, :])
        # ---- build lhs table [N_OT,128] ----
        NWO = (N_OT + W - 1) // W
        for w in range(NWO):
            n0 = w * W
            n1 = min(N_OT, n0 + W)
            m = n1 - n0
            aT = tabp.tile([64, 128], dt.bfloat16, tag="bT")
            nc.sync.dma_start(out=aT[:, :m], in_=otT_ext[:, n0:n1])
            ps = psA.tile([128, 64], dt.float32, tag="tps")
            nc.tensor.matmul(out=ps[:m, :], lhsT=aT[:, :m], rhs=Wo_t[:], start=True, stop=True)
            sb = tabp.tile([128, 64], dt.bfloat16, tag="tsb")
            nc.vector.tensor_copy(out=sb[:m, :], in_=ps[:m, :])
            nc.sync.dma_start(out=ltab[n0:n1, 0:64], in_=sb[:m, :])

        # ---- main loop: gather blocks, per-tile message + window scatter ----
        # track PSUM window accumulation: flush when window changes
        cur = {"w": None, "seg": None, "ps": None, "n0": None}

        def flush():
            if cur["ps"] is not None:
                wv = cur["w"]
                nc.vector.tensor_tensor(out=acc[:, wv, :], in0=acc[:, wv, :],
                                        in1=cur["ps"][:], op=mybir.AluOpType.add)
                cur["ps"] = None

        t_global = 0
        for b in range(NBLK):
            segb = 0 if (b * BLK) < TA_tok else 1
            tab_ap = ltab[0:SEG_SPLIT, :] if segb == 0 else ltab[SEG_SPLIT:N_OT, :]
            gl = gp.tile([128, TPB, 128], dt.bfloat16, tag="gl")
            nc.gpsimd.dma_gather(gl[:], tab_ap, lix[:, b * (BLK // 16):(b + 1) * (BLK // 16)],
                                 BLK, BLK, 128, queue_num=b % 4)
            gr = gp.tile([128, TPB, 128], dt.bfloat16, tag="gr")
            nc.gpsimd.dma_gather(gr[:], rtab[:], rix[:, b * (BLK // 16):(b + 1) * (BLK // 16)],
                                 BLK, BLK, 128, queue_num=(b + 2) % 4)
            msum = wk.tile([128, TPB, 64], dt.bfloat16, tag="msum")
            nc.vector.tensor_tensor(out=msum[:], in0=gl[:, :, 0:64], in1=gr[:, :, 0:64],
                                    op=mybir.AluOpType.add)
            m2b = wk.tile([128, TPB, 64], dt.bfloat16, tag="m2b")
            ohs = []
            for i in range(TPB):
                t = t_global + i
                oh = ohp.tile([128, 128], dt.bfloat16, tag="oh", name=f"oh{t}")
                nc.vector.tensor_scalar(out=oh[:], in0=iota[:], scalar1=rjl[:, t:t + 1],
                                        scalar2=None, op0=mybir.AluOpType.is_equal)
                ohs.append(oh)
                wwe = wk.tile([128, 64], dt.bfloat16, tag="wwe")
                nc.vector.tensor_scalar(out=wwe[:], in0=WeR[:], scalar1=wg[:, t:t + 1],
                                        scalar2=None, op0=mybir.AluOpType.mult)
                nc.vector.tensor_tensor(out=m2b[:, i, :], in0=msum[:, i, :], in1=wwe[:],
                                        op=mybir.AluOpType.add)
            mrb = wk.tile([128, TPB, 64], dt.bfloat16, tag="mrb")
            nc.scalar.activation(out=mrb[:], in_=m2b[:],
                                 func=mybir.ActivationFunctionType.Lrelu, alpha=0.01)
            for i in range(TPB):
                t = t_global
                t_global += 1
                sg, wv = sched[t]
                if wv < 0:
                    continue
                if cur["w"] != wv or cur["seg"] != sg:
                    flush()
                    cur["w"], cur["seg"] = wv, sg
                    cur["ps"] = psW.tile([128, 64], dt.float32, tag="psw", name=f"psw{t}")
                    cur["first"] = True
                nc.tensor.matmul(out=cur["ps"][:], lhsT=ohs[i][:], rhs=mrb[:, i, :],
                                 start=cur["first"], stop=False)
                cur["first"] = False
        flush()

        # ---- final: y = S@M1 + counts x v1 + 1 x bout + input@W2 ----
        M1t = cpool.tile([64, 64], dt.bfloat16)
        nc.sync.dma_start(out=M1t[:], in_=M1_ext[:])
        W2t = cpool.tile([64, 64], dt.bfloat16)
        nc.sync.dma_start(out=W2t[:], in_=W2_ext[:])
        vbt = cpool.tile([2, 64], dt.bfloat16)
        nc.sync.dma_start(out=vbt[:], in_=vb_ext[:])
        cntr = cpool.tile([2, NPC], dt.bfloat16)
        nc.sync.dma_start(out=cntr[:], in_=cnts_ext[:])
        ident = cpool.tile([128, 128], dt.bfloat16)
        from concourse.masks import make_identity
        make_identity(nc, ident[:])

        for w in range(NW):
            n0 = w * W
            n1 = min(NPC, n0 + W)
            m = n1 - n0
            # S_w bf16 and its transpose
            swb = wk.tile([128, 64], dt.bfloat16, tag="swb")
            nc.vector.tensor_copy(out=swb[:], in_=acc[:, w, :])
            tps = psA.tile([128, 128], dt.bfloat16, tag="fps")
            nc.tensor.transpose(out=tps[0:64, :], in_=swb[:], identity=ident[:])
            swT = wk.tile([64, 128], dt.bfloat16, tag="swT")
            nc.vector.tensor_copy(out=swT[:], in_=tps[0:64, :])
            inw = wk.tile([65, 128], dt.bfloat16, tag="inw")
            nc.sync.dma_start(out=inw[:, :m], in_=inT_ext[:, n0:n1])
            ops = psA.tile([128, 64], dt.float32, tag="ops")
            nc.tensor.matmul(out=ops[:m, :], lhsT=swT[:, :m], rhs=M1t[:], start=True, stop=False)
            nc.tensor.matmul(out=ops[:m, :], lhsT=inw[0:64, :m], rhs=W2t[:], start=False, stop=False)
            nc.tensor.matmul(out=ops[:m, :], lhsT=cntr[:, n0:n1], rhs=vbt[:], start=False, stop=True)
            ob = wk.tile([128, 64], dt.float32, tag="ob")
            nc.vector.tensor_copy(out=ob[:m, :], in_=ops[:m, :])
            nc.sync.dma_start(out=y_ext[n0:n1, :], in_=ob[:m, :])

    nc.compile()

    # ---------------- host-side in_maps ----------------
    W1 = Wout[:64]; W2 = Wout[64:]
    M1 = (Wf @ W1).astype(np.float32)
    v1 = (bf @ W1).astype(np.float32)
    vb = np.stack([v1, bout]).astype(bf16)
    iota_np = np.tile(np.arange(128, dtype=np.float32)[None, :], (128, 1)).astype(bf16)
    WeR_np = np.tile(We[None, :], (128, 1)).astype(bf16)
    WiB_np = np.concatenate([Wi, bi[None, :]], 0).astype(bf16)

    in_maps = []
    for c in range(NC):
        sl = input[c * NPC:(c + 1) * NPC]
        inT = np.concatenate([sl.T, np.ones((1, NPC), np.float32)], 0).astype(bf16)
        in_maps.append({
            "inT": np.ascontiguousarray(inT),
            "otT": np.ascontiguousarray(other.T).astype(bf16),
            "WiB": WiB_np, "Wo_": Wo.astype(bf16),
            "M1_": M1.astype(bf16), "W2_": W2.astype(bf16), "vb_": vb,
            "cnts": np.stack([counts[c], np.ones(NPC, np.float32)]).astype(bf16),
            "WeR": WeR_np, "iot": iota_np,
            "rix": _wrap16(rhs_idx[c]), "lix": _wrap16(lhs_idx[c]),
            "rjl": grid_pt(rjl_grid[c], np.float32),
            "wg": grid_pt(w_grid[c], np.float32),
        })

    import os
    res = run_bass_kernel_spmd(nc, in_maps, list(range(NC)),
                               trace=bool(os.environ.get("KTRACE")))
    if os.environ.get("KTRACE") and res.exec_time_ns:
        print(f"HW exec time: {res.exec_time_ns} ns")
    out = np.concatenate([res.results[c]["y"] for c in range(NC)], 0)
    return out.astype(np.float32)



# revision 10
# speedup vs baseline: 1.5558x; 1.4363x over previous
"""BipartiteGConv Trainium2 kernel (8 NeuronCores, edge/node sharding).

Math (see reference):
  rhs = input @ Wi + bi            [N_IN, D]
  lhs = other @ Wo                 [N_OT, D]
  msg = lrelu(rhs[rj] + lhs[lj] + w*We) per edge
  S   = segment_sum(msg, rj)       [N_IN, D]
  out = concat([S @ Wf + bf, input]) @ Wout + bout
      = S @ (Wf@W1) + counts x (bf@W1) + input @ W2 + bout   (W1=Wout[:D], W2=Wout[D:])

Sharding: nodes (rj ranges of 12500) across 8 cores; each core owns all
edges targeting its range.  Slots ordered by (lj-segment, rj-window of 128
nodes), padded per (window, seg) to uniform tile counts across cores
(SPMD).  Gathers via dma_gather (bf16-padded 256B rows); segment-sum via
per-tile onehot matmul accumulated in PSUM per window.
"""
import sys
sys.path.insert(0, "/opt/trn_rl_repo")
import numpy as np
import ml_dtypes

N_IN, N_OT, E, D = 100000, 50000, 1000000, 64
NC = 8
NPC = N_IN // NC            # nodes per core
W = 128                     # window size (nodes)
NW = (NPC + W - 1) // W     # windows per core
SEG_SPLIT = 32768           # lhs table A/B split (int16 idx limit)
BLK = 1024                  # gather tokens per instruction
PADV = 999.0                # rj_local value for pad slots (onehot -> 0)


def _wrap16(a):
    # token i -> [i % 16, i // 16], replicated to 128 partitions
    n = a.shape[0]
    assert n % 16 == 0
    return np.tile(a.reshape(n // 16, 16).T, (8, 1)).copy()


def kernel(input, other, rj, lj, weights, Wi, bi, Wo, We, Wf, bf, Wout, bout):
    import concourse.bass as bass
    import concourse.bacc as bacc
    import concourse.mybir as mybir
    import concourse.tile as tile
    from concourse.bass_utils import run_bass_kernel_spmd
    from contextlib import ExitStack

    input = np.asarray(input, np.float32)
    other = np.asarray(other, np.float32)
    rj = np.asarray(rj).astype(np.int64)
    lj = np.asarray(lj).astype(np.int64)
    weights = np.asarray(weights, np.float32).reshape(-1)
    Wi = np.asarray(Wi, np.float32); bi = np.asarray(bi, np.float32)
    Wo = np.asarray(Wo, np.float32); We = np.asarray(We, np.float32).reshape(-1)
    Wf = np.asarray(Wf, np.float32); bf = np.asarray(bf, np.float32)
    Wout = np.asarray(Wout, np.float32); bout = np.asarray(bout, np.float32)

    bf16 = ml_dtypes.bfloat16

    # ---------------- host index prep (per core) ----------------
    core_of = rj // NPC
    order0 = np.argsort(core_of, kind="stable")
    # per (core, seg, window) edge lists
    tiles_per = np.zeros((NC, 2, NW), np.int64)
    core_data = []
    for c in range(NC):
        sel = order0[np.searchsorted(core_of, c, side="left", sorter=order0):
                     np.searchsorted(core_of, c, side="right", sorter=order0)]
        sel = order0[core_of[order0] == c] if False else sel
        rjl_all = rj[sel] - c * NPC
        win = rjl_all // W
        seg = (lj[sel] >= SEG_SPLIT).astype(np.int64)
        key = seg * NW + win
        o2 = np.argsort(key, kind="stable")
        core_data.append((sel[o2], (rjl_all % W)[o2], key[o2]))
        cnt = np.bincount(key[o2], minlength=2 * NW).reshape(2, NW)
        tiles_per[c] = (cnt + 127) // 128
    TW = tiles_per.max(axis=0)          # uniform tiles per (seg, window)
    # segment A tile count rounded so segment boundary is BLK-aligned
    TA = int(TW[0].sum()); TB = int(TW[1].sum())
    padA = (-TA) % (BLK // 128)
    padB = (-(TA + padA + TB)) % (BLK // 128)
    # tile schedule: list of (seg, window) per tile, with pad tiles (seg, -1)
    sched = []
    for w in range(NW):
        sched += [(0, w)] * int(TW[0, w])
    sched += [(0, -1)] * padA
    for w in range(NW):
        sched += [(1, w)] * int(TW[1, w])
    sched += [(1, -1)] * padB
    T = len(sched)
    S = T * 128
    TA_tok = (TA + padA) * 128          # segment A token count (BLK-aligned)

    rhs_idx = np.zeros((NC, S), np.int16)
    lhs_idx = np.zeros((NC, S), np.int16)
    rjl_grid = np.full((NC, S), PADV, np.float32)
    w_grid = np.zeros((NC, S), np.float32)
    counts = np.zeros((NC, NPC), np.float32)
    for c in range(NC):
        sel, rjl_loc, key = core_data[c]
        counts[c] = np.bincount(rj[sel] - c * NPC, minlength=NPC)
        # slot positions: walk schedule, fill each (seg,window) group
        pos = 0
        ptr = 0  # pointer into sel
        for (sg, w) in sched:
            if w < 0:
                pos += 128
                continue
            k = sg * NW + w
            # edges for this (seg, window) in this core
            lo = np.searchsorted(key, k, side="left")
            hi = np.searchsorted(key, k, side="right")
            ne = hi - lo
            ntile = int(TW[sg, w])
            # how many already consumed for this key from earlier tiles
            # (groups are contiguous; fill greedily tile by tile)
            take = min(128, ne - (ptr if False else 0))
            # simpler: fill the whole group at its first tile encounter
            if lo != hi:
                e0 = np.arange(lo, hi)
                p0 = pos  # this is the first tile of the group only if we track it
            pos += 128
        # vectorized fill instead (group-contiguous):
        pos_of_group = {}
        p = 0
        for (sg, w) in sched:
            if w >= 0 and (sg, w) not in pos_of_group:
                pos_of_group[(sg, w)] = p
            p += 128
        for sg in range(2):
            for w in range(NW):
                k = sg * NW + w
                lo = np.searchsorted(key, k, side="left")
                hi = np.searchsorted(key, k, side="right")
                if lo == hi:
                    continue
                base = pos_of_group[(sg, w)]
                idxs = np.arange(base, base + (hi - lo))
                ee = sel[lo:hi]
                rhs_idx[c, idxs] = (rj[ee] - c * NPC).astype(np.int16)
                lv = lj[ee] - sg * SEG_SPLIT
                lhs_idx[c, idxs] = lv.astype(np.int16)
                rjl_grid[c, idxs] = rjl_loc[lo:hi].astype(np.float32)
                w_grid[c, idxs] = weights[ee]

    # slot s maps to (p, t) = (s % 128, s // 128) [dma_gather token layout]
    def grid_pt(a, dt):
        return np.ascontiguousarray(a.reshape(T, 128).T).astype(dt)

    # ---------------- build bass kernel ----------------
    dt = mybir.dt
    nc = bacc.Bacc("TRN2", target_bir_lowering=False, debug=False,
                   num_devices=NC, num_swdge_queues=4)

    inT_ext = nc.dram_tensor("inT", [65, NPC], dt.bfloat16, kind="ExternalInput").ap()
    otT_ext = nc.dram_tensor("otT", [64, N_OT], dt.bfloat16, kind="ExternalInput").ap()
    WiB_ext = nc.dram_tensor("WiB", [65, 64], dt.bfloat16, kind="ExternalInput").ap()
    Wo_ext = nc.dram_tensor("Wo_", [64, 64], dt.bfloat16, kind="ExternalInput").ap()
    M1_ext = nc.dram_tensor("M1_", [64, 64], dt.bfloat16, kind="ExternalInput").ap()
    W2_ext = nc.dram_tensor("W2_", [64, 64], dt.bfloat16, kind="ExternalInput").ap()
    vb_ext = nc.dram_tensor("vb_", [2, 64], dt.bfloat16, kind="ExternalInput").ap()
    cnts_ext = nc.dram_tensor("cnts", [2, NPC], dt.bfloat16, kind="ExternalInput").ap()
    WeR_ext = nc.dram_tensor("WeR", [128, 64], dt.bfloat16, kind="ExternalInput").ap()
    iota_ext = nc.dram_tensor("iot", [128, 128], dt.bfloat16, kind="ExternalInput").ap()
    rix_ext = nc.dram_tensor("rix", [128, S // 16], dt.int16, kind="ExternalInput").ap()
    lix_ext = nc.dram_tensor("lix", [128, S // 16], dt.int16, kind="ExternalInput").ap()
    rjl_ext = nc.dram_tensor("rjl", [128, T], dt.bfloat16, kind="ExternalInput").ap()
    wg_ext = nc.dram_tensor("wg", [128, T], dt.bfloat16, kind="ExternalInput").ap()
    y_ext = nc.dram_tensor("y", [NPC, 64], dt.float32, kind="ExternalOutput").ap()

    rtab = nc.dram_tensor("rtab", [NPC, 128], dt.bfloat16).ap()
    ltab = nc.dram_tensor("ltab", [N_OT, 128], dt.bfloat16).ap()

    NBLK = S // BLK
    TPB = BLK // 128  # tiles per block = 8

    with tile.TileContext(nc) as tc, ExitStack() as ctx:
        cpool = ctx.enter_context(tc.tile_pool(name="const", bufs=1))
        tabp = ctx.enter_context(tc.tile_pool(name="tab", bufs=3))
        gp = ctx.enter_context(tc.tile_pool(name="gath", bufs=6))
        wk = ctx.enter_context(tc.tile_pool(name="work", bufs=4))
        ohp = ctx.enter_context(tc.tile_pool(name="ohp", bufs=2))
        psA = ctx.enter_context(tc.tile_pool(name="psA", bufs=2, space="PSUM"))
        psW = ctx.enter_context(tc.tile_pool(name="psW", bufs=2, space="PSUM"))
        accp = ctx.enter_context(tc.tile_pool(name="acc", bufs=1))

        iota = cpool.tile([128, 128], dt.bfloat16)
        nc.sync.dma_start(out=iota[:], in_=iota_ext[:])
        WeR = cpool.tile([128, 64], dt.bfloat16)
        nc.sync.dma_start(out=WeR[:], in_=WeR_ext[:])
        WiB = cpool.tile([65, 64], dt.bfloat16)
        nc.sync.dma_start(out=WiB[:], in_=WiB_ext[:])
        Wo_t = cpool.tile([64, 64], dt.bfloat16)
        nc.sync.dma_start(out=Wo_t[:], in_=Wo_ext[:])
        rjl = cpool.tile([128, T], dt.bfloat16)
        nc.sync.dma_start(out=rjl[:], in_=rjl_ext[:])
        wg = cpool.tile([128, T], dt.bfloat16)
        nc.sync.dma_start(out=wg[:], in_=wg_ext[:])
        rix = cpool.tile([128, S // 16], dt.int16)
        nc.sync.dma_start(out=rix[:], in_=rix_ext[:])
        lix = cpool.tile([128, S // 16], dt.int16)
        nc.sync.dma_start(out=lix[:], in_=lix_ext[:])

        acc = accp.tile([128, NW, 64], dt.float32)
        nc.vector.memset(acc[:], 0.0)

        # ---- build rhs table [NPC,128] (cols 0:64 = input@Wi+bi, bf16) ----
        for w in range(NW):
            n0 = w * W
            n1 = min(NPC, n0 + W)
            m = n1 - n0
            aT = tabp.tile([65, 128], dt.bfloat16, tag="aT")
            nc.sync.dma_start(out=aT[:, :m], in_=inT_ext[:, n0:n1])
            ps = psA.tile([128, 64], dt.float32, tag="tps")
            nc.tensor.matmul(out=ps[:m, :], lhsT=aT[:, :m], rhs=WiB[:], start=True, stop=True)
            sb = tabp.tile([128, 64], dt.bfloat16, tag="tsb")
            nc.vector.tensor_copy(out=sb[:m, :], in_=ps[:m, :])
            nc.sync.dma_start(out=rtab[n0:n1, 0:64], in_=sb[:m, :])
        # ---- build lhs table [N_OT,128] ----
        NWO = (N_OT + W - 1) // W
        for w in range(NWO):
            n0 = w * W
            n1 = min(N_OT, n0 + W)
            m = n1 - n0
            aT = tabp.tile([64, 128], dt.bfloat16, tag="bT")
            nc.sync.dma_start(out=aT[:, :m], in_=otT_ext[:, n0:n1])
            ps = psA.tile([128, 64], dt.float32, tag="tps")
            nc.tensor.matmul(out=ps[:m, :], lhsT=aT[:, :m], rhs=Wo_t[:], start=True, stop=True)
            sb = tabp.tile([128, 64], dt.bfloat16, tag="tsb")
            nc.vector.tensor_copy(out=sb[:m, :], in_=ps[:m, :])
            nc.sync.dma_start(out=ltab[n0:n1, 0:64], in_=sb[:m, :])

        # ---- main loop: gather blocks, per-tile message + window scatter ----
        # track PSUM window accumulation: flush when window changes
        cur = {"w": None, "seg": None, "ps": None, "n0": None}

        def flush():
            if cur["ps"] is not None:
                wv = cur["w"]
                nc.vector.tensor_tensor(out=acc[:, wv, :], in0=acc[:, wv, :],
                                        in1=cur["ps"][:], op=mybir.AluOpType.add)
                cur["ps"] = None

        t_global = 0
        for b in range(NBLK):
            segb = 0 if (b * BLK) < TA_tok else 1
            tab_ap = ltab[0:SEG_SPLIT, :] if segb == 0 else ltab[SEG_SPLIT:N_OT, :]
            gl = gp.tile([128, TPB, 128], dt.bfloat16, tag="gl")
            nc.gpsimd.dma_gather(gl[:], tab_ap, lix[:, b * (BLK // 16):(b + 1) * (BLK // 16)],
                                 BLK, BLK, 128, queue_num=b % 4)
            gr = gp.tile([128, TPB, 128], dt.bfloat16, tag="gr")
            nc.gpsimd.dma_gather(gr[:], rtab[:], rix[:, b * (BLK // 16):(b + 1) * (BLK // 16)],
                                 BLK, BLK, 128, queue_num=(b + 2) % 4)
            msum = wk.tile([128, TPB, 64], dt.bfloat16, tag="msum")
            nc.vector.tensor_tensor(out=msum[:], in0=gl[:, :, 0:64], in1=gr[:, :, 0:64],
                                    op=mybir.AluOpType.add)
            t0 = t_global
            oh_all = ohp.tile([128, TPB, 128], dt.bfloat16, tag="ohall")
            rjl_rep = wk.tile([128, TPB, 128], dt.bfloat16, tag="rjlrep")
            nc.vector.tensor_copy(
                out=rjl_rep[:],
                in_=rjl[:, t0:t0 + TPB, None].to_broadcast([128, TPB, 128]))
            nc.vector.tensor_tensor(
                out=oh_all[:], in0=iota[:, None, :].to_broadcast([128, TPB, 128]),
                in1=rjl_rep[:], op=mybir.AluOpType.is_equal)
            ohs = [oh_all[:, i, :] for i in range(TPB)]
            w_rep = wk.tile([128, TPB, 64], dt.bfloat16, tag="wrep")
            nc.vector.tensor_copy(
                out=w_rep[:],
                in_=wg[:, t0:t0 + TPB, None].to_broadcast([128, TPB, 64]))
            wwe_all = wk.tile([128, TPB, 64], dt.bfloat16, tag="wweall")
            nc.vector.tensor_tensor(
                out=wwe_all[:], in0=w_rep[:],
                in1=WeR[:, None, :].to_broadcast([128, TPB, 64]),
                op=mybir.AluOpType.mult)
            m2b = wk.tile([128, TPB, 64], dt.bfloat16, tag="m2b")
            nc.vector.tensor_tensor(out=m2b[:], in0=msum[:], in1=wwe_all[:],
                                    op=mybir.AluOpType.add)
            mrb = wk.tile([128, TPB, 64], dt.bfloat16, tag="mrb")
            nc.scalar.activation(out=mrb[:], in_=m2b[:],
                                 func=mybir.ActivationFunctionType.Lrelu, alpha=0.01)
            for i in range(TPB):
                t = t_global
                t_global += 1
                sg, wv = sched[t]
                if wv < 0:
                    continue
                if cur["w"] != wv or cur["seg"] != sg:
                    flush()
                    cur["w"], cur["seg"] = wv, sg
                    cur["ps"] = psW.tile([128, 64], dt.float32, tag="psw", name=f"psw{t}")
                    cur["first"] = True
                nc.tensor.matmul(out=cur["ps"][:], lhsT=ohs[i], rhs=mrb[:, i, :],
                                 start=cur["first"], stop=False)
                cur["first"] = False
        flush()

        # ---- final: y = S@M1 + counts x v1 + 1 x bout + input@W2 ----
        M1t = cpool.tile([64, 64], dt.bfloat16)
        nc.sync.dma_start(out=M1t[:], in_=M1_ext[:])
        W2t = cpool.tile([64, 64], dt.bfloat16)
        nc.sync.dma_start(out=W2t[:], in_=W2_ext[:])
        vbt = cpool.tile([2, 64], dt.bfloat16)
        nc.sync.dma_start(out=vbt[:], in_=vb_ext[:])
        cntr = cpool.tile([2, NPC], dt.bfloat16)
        nc.sync.dma_start(out=cntr[:], in_=cnts_ext[:])
        ident = cpool.tile([128, 128], dt.bfloat16)
        from concourse.masks import make_identity
        make_identity(nc, ident[:])

        for w in range(NW):
            n0 = w * W
            n1 = min(NPC, n0 + W)
            m = n1 - n0
            # S_w bf16 and its transpose
            swb = wk.tile([128, 64], dt.bfloat16, tag="swb")
            nc.vector.tensor_copy(out=swb[:], in_=acc[:, w, :])
            tps = psA.tile([128, 128], dt.bfloat16, tag="fps")
            nc.tensor.transpose(out=tps[0:64, :], in_=swb[:], identity=ident[:])
            swT = wk.tile([64, 128], dt.bfloat16, tag="swT")
            nc.vector.tensor_copy(out=swT[:], in_=tps[0:64, :])
            inw = wk.tile([65, 128], dt.bfloat16, tag="inw")
            nc.sync.dma_start(out=inw[:, :m], in_=inT_ext[:, n0:n1])
            ops = psA.tile([128, 64], dt.float32, tag="ops")
            nc.tensor.matmul(out=ops[:m, :], lhsT=swT[:, :m], rhs=M1t[:], start=True, stop=False)
            nc.tensor.matmul(out=ops[:m, :], lhsT=inw[0:64, :m], rhs=W2t[:], start=False, stop=False)
            nc.tensor.matmul(out=ops[:m, :], lhsT=cntr[:, n0:n1], rhs=vbt[:], start=False, stop=True)
            ob = wk.tile([128, 64], dt.float32, tag="ob")
            nc.vector.tensor_copy(out=ob[:m, :], in_=ops[:m, :])
            nc.sync.dma_start(out=y_ext[n0:n1, :], in_=ob[:m, :])

    nc.compile()

    # ---------------- host-side in_maps ----------------
    W1 = Wout[:64]; W2 = Wout[64:]
    M1 = (Wf @ W1).astype(np.float32)
    v1 = (bf @ W1).astype(np.float32)
    vb = np.stack([v1, bout]).astype(bf16)
    iota_np = np.tile(np.arange(128, dtype=np.float32)[None, :], (128, 1)).astype(bf16)
    WeR_np = np.tile(We[None, :], (128, 1)).astype(bf16)
    WiB_np = np.concatenate([Wi, bi[None, :]], 0).astype(bf16)

    in_maps = []
    for c in range(NC):
        sl = input[c * NPC:(c + 1) * NPC]
        inT = np.concatenate([sl.T, np.ones((1, NPC), np.float32)], 0).astype(bf16)
        in_maps.append({
            "inT": np.ascontiguousarray(inT),
            "otT": np.ascontiguousarray(other.T).astype(bf16),
            "WiB": WiB_np, "Wo_": Wo.astype(bf16),
            "M1_": M1.astype(bf16), "W2_": W2.astype(bf16), "vb_": vb,
            "cnts": np.stack([counts[c], np.ones(NPC, np.float32)]).astype(bf16),
            "WeR": WeR_np, "iot": iota_np,
            "rix": _wrap16(rhs_idx[c]), "lix": _wrap16(lhs_idx[c]),
            "rjl": grid_pt(rjl_grid[c], bf16),
            "wg": grid_pt(w_grid[c], bf16),
        })

    import os
    res = run_bass_kernel_spmd(nc, in_maps, list(range(NC)),
                               trace=bool(os.environ.get("KTRACE")))
    if os.environ.get("KTRACE") and res.exec_time_ns:
        print(f"HW exec time: {res.exec_time_ns} ns")
    out = np.concatenate([res.results[c]["y"] for c in range(NC)], 0)
    return out.astype(np.float32)



# revision 11
# speedup vs baseline: 1.9200x; 1.2341x over previous
"""BipartiteGConv Trainium2 kernel (8 NeuronCores, edge/node sharding).

Math (see reference):
  rhs = input @ Wi + bi            [N_IN, D]
  lhs = other @ Wo                 [N_OT, D]
  msg = lrelu(rhs[rj] + lhs[lj] + w*We) per edge
  S   = segment_sum(msg, rj)       [N_IN, D]
  out = concat([S @ Wf + bf, input]) @ Wout + bout
      = S @ (Wf@W1) + counts x (bf@W1) + input @ W2 + bout   (W1=Wout[:D], W2=Wout[D:])

Sharding: nodes (rj ranges of 12500) across 8 cores; each core owns all
edges targeting its range.  Slots ordered by (lj-segment, rj-window of 128
nodes), padded per (window, seg) to uniform tile counts across cores
(SPMD).  Gathers via dma_gather (bf16-padded 256B rows); segment-sum via
per-tile onehot matmul accumulated in PSUM per window.
"""
import sys
sys.path.insert(0, "/opt/trn_rl_repo")
import numpy as np
import ml_dtypes

N_IN, N_OT, E, D = 100000, 50000, 1000000, 64
NC = 8
NPC = N_IN // NC            # nodes per core
W = 128                     # window size (nodes)
NW = (NPC + W - 1) // W     # windows per core
SEG_SPLIT = 32768           # lhs table A/B split (int16 idx limit)
BLK = 1024                  # gather tokens per instruction
PADV = 999.0                # rj_local value for pad slots (onehot -> 0)


def _wrap16(a):
    # token i -> [i % 16, i // 16], replicated to 128 partitions
    n = a.shape[0]
    assert n % 16 == 0
    return np.tile(a.reshape(n // 16, 16).T, (8, 1)).copy()


def kernel(input, other, rj, lj, weights, Wi, bi, Wo, We, Wf, bf, Wout, bout):
    import concourse.bass as bass
    import concourse.bacc as bacc
    import concourse.mybir as mybir
    import concourse.tile as tile
    from concourse.bass_utils import run_bass_kernel_spmd
    from contextlib import ExitStack

    input = np.asarray(input, np.float32)
    other = np.asarray(other, np.float32)
    rj = np.asarray(rj).astype(np.int64)
    lj = np.asarray(lj).astype(np.int64)
    weights = np.asarray(weights, np.float32).reshape(-1)
    Wi = np.asarray(Wi, np.float32); bi = np.asarray(bi, np.float32)
    Wo = np.asarray(Wo, np.float32); We = np.asarray(We, np.float32).reshape(-1)
    Wf = np.asarray(Wf, np.float32); bf = np.asarray(bf, np.float32)
    Wout = np.asarray(Wout, np.float32); bout = np.asarray(bout, np.float32)

    bf16 = ml_dtypes.bfloat16

    # ---------------- host index prep (per core) ----------------
    core_of = rj // NPC
    order0 = np.argsort(core_of, kind="stable")
    # per (core, seg, window) edge lists
    tiles_per = np.zeros((NC, 2, NW), np.int64)
    core_data = []
    for c in range(NC):
        sel = order0[np.searchsorted(core_of, c, side="left", sorter=order0):
                     np.searchsorted(core_of, c, side="right", sorter=order0)]
        sel = order0[core_of[order0] == c] if False else sel
        rjl_all = rj[sel] - c * NPC
        win = rjl_all // W
        seg = (lj[sel] >= SEG_SPLIT).astype(np.int64)
        key = seg * NW + win
        o2 = np.argsort(key, kind="stable")
        core_data.append((sel[o2], (rjl_all % W)[o2], key[o2]))
        cnt = np.bincount(key[o2], minlength=2 * NW).reshape(2, NW)
        tiles_per[c] = (cnt + 127) // 128
    TW = tiles_per.max(axis=0)          # uniform tiles per (seg, window)
    # segment A tile count rounded so segment boundary is BLK-aligned
    TA = int(TW[0].sum()); TB = int(TW[1].sum())
    padA = (-TA) % (BLK // 128)
    padB = (-(TA + padA + TB)) % (BLK // 128)
    # tile schedule: list of (seg, window) per tile, with pad tiles (seg, -1)
    sched = []
    for w in range(NW):
        sched += [(0, w)] * int(TW[0, w])
    sched += [(0, -1)] * padA
    for w in range(NW):
        sched += [(1, w)] * int(TW[1, w])
    sched += [(1, -1)] * padB
    T = len(sched)
    S = T * 128
    TA_tok = (TA + padA) * 128          # segment A token count (BLK-aligned)

    rhs_idx = np.zeros((NC, S), np.int16)
    lhs_idx = np.zeros((NC, S), np.int16)
    rjl_grid = np.full((NC, S), PADV, np.float32)
    w_grid = np.zeros((NC, S), np.float32)
    counts = np.zeros((NC, NPC), np.float32)
    for c in range(NC):
        sel, rjl_loc, key = core_data[c]
        counts[c] = np.bincount(rj[sel] - c * NPC, minlength=NPC)
        # slot positions: walk schedule, fill each (seg,window) group
        pos = 0
        ptr = 0  # pointer into sel
        for (sg, w) in sched:
            if w < 0:
                pos += 128
                continue
            k = sg * NW + w
            # edges for this (seg, window) in this core
            lo = np.searchsorted(key, k, side="left")
            hi = np.searchsorted(key, k, side="right")
            ne = hi - lo
            ntile = int(TW[sg, w])
            # how many already consumed for this key from earlier tiles
            # (groups are contiguous; fill greedily tile by tile)
            take = min(128, ne - (ptr if False else 0))
            # simpler: fill the whole group at its first tile encounter
            if lo != hi:
                e0 = np.arange(lo, hi)
                p0 = pos  # this is the first tile of the group only if we track it
            pos += 128
        # vectorized fill instead (group-contiguous):
        pos_of_group = {}
        p = 0
        for (sg, w) in sched:
            if w >= 0 and (sg, w) not in pos_of_group:
                pos_of_group[(sg, w)] = p
            p += 128
        for sg in range(2):
            for w in range(NW):
                k = sg * NW + w
                lo = np.searchsorted(key, k, side="left")
                hi = np.searchsorted(key, k, side="right")
                if lo == hi:
                    continue
                base = pos_of_group[(sg, w)]
                idxs = np.arange(base, base + (hi - lo))
                ee = sel[lo:hi]
                rhs_idx[c, idxs] = (rj[ee] - c * NPC).astype(np.int16)
                lv = lj[ee] - sg * SEG_SPLIT
                lhs_idx[c, idxs] = lv.astype(np.int16)
                rjl_grid[c, idxs] = rjl_loc[lo:hi].astype(np.float32)
                w_grid[c, idxs] = weights[ee]

    # slot s maps to (p, t) = (s % 128, s // 128) [dma_gather token layout]
    def grid_pt(a, dt):
        return np.ascontiguousarray(a.reshape(T, 128).T).astype(dt)

    # ---------------- build bass kernel ----------------
    dt = mybir.dt
    nc = bacc.Bacc("TRN2", target_bir_lowering=False, debug=False,
                   num_devices=NC, num_swdge_queues=4)

    inT_ext = nc.dram_tensor("inT", [65, NPC], dt.bfloat16, kind="ExternalInput").ap()
    otT_ext = nc.dram_tensor("otT", [64, N_OT], dt.bfloat16, kind="ExternalInput").ap()
    WiB_ext = nc.dram_tensor("WiB", [65, 64], dt.bfloat16, kind="ExternalInput").ap()
    Wo_ext = nc.dram_tensor("Wo_", [64, 64], dt.bfloat16, kind="ExternalInput").ap()
    M1_ext = nc.dram_tensor("M1_", [64, 64], dt.bfloat16, kind="ExternalInput").ap()
    W2_ext = nc.dram_tensor("W2_", [64, 64], dt.bfloat16, kind="ExternalInput").ap()
    vb_ext = nc.dram_tensor("vb_", [2, 64], dt.bfloat16, kind="ExternalInput").ap()
    cnts_ext = nc.dram_tensor("cnts", [2, NPC], dt.bfloat16, kind="ExternalInput").ap()
    WeR_ext = nc.dram_tensor("WeR", [128, 64], dt.bfloat16, kind="ExternalInput").ap()
    iota_ext = nc.dram_tensor("iot", [128, 128], dt.bfloat16, kind="ExternalInput").ap()
    rix_ext = nc.dram_tensor("rix", [128, S // 16], dt.int16, kind="ExternalInput").ap()
    lix_ext = nc.dram_tensor("lix", [128, S // 16], dt.int16, kind="ExternalInput").ap()
    rjl_ext = nc.dram_tensor("rjl", [128, T], dt.bfloat16, kind="ExternalInput").ap()
    wg_ext = nc.dram_tensor("wg", [128, T], dt.bfloat16, kind="ExternalInput").ap()
    y_ext = nc.dram_tensor("y", [NPC, 64], dt.float32, kind="ExternalOutput").ap()

    rtab = nc.dram_tensor("rtab", [NPC, 128], dt.bfloat16).ap()
    ltab = nc.dram_tensor("ltab", [N_OT, 128], dt.bfloat16).ap()

    NBLK = S // BLK
    TPB = BLK // 128  # tiles per block = 8

    with tile.TileContext(nc) as tc, ExitStack() as ctx:
        cpool = ctx.enter_context(tc.tile_pool(name="const", bufs=1))
        tabp = ctx.enter_context(tc.tile_pool(name="tab", bufs=3))
        gp = ctx.enter_context(tc.tile_pool(name="gath", bufs=6))
        wk = ctx.enter_context(tc.tile_pool(name="work", bufs=4))
        ohp = ctx.enter_context(tc.tile_pool(name="ohp", bufs=2))
        psA = ctx.enter_context(tc.tile_pool(name="psA", bufs=2, space="PSUM"))
        psW = ctx.enter_context(tc.tile_pool(name="psW", bufs=2, space="PSUM"))
        accp = ctx.enter_context(tc.tile_pool(name="acc", bufs=1))

        iota = cpool.tile([128, 128], dt.bfloat16)
        nc.sync.dma_start(out=iota[:], in_=iota_ext[:])
        WeR = cpool.tile([128, 64], dt.bfloat16)
        nc.sync.dma_start(out=WeR[:], in_=WeR_ext[:])
        WiB = cpool.tile([65, 64], dt.bfloat16)
        nc.sync.dma_start(out=WiB[:], in_=WiB_ext[:])
        Wo_t = cpool.tile([64, 64], dt.bfloat16)
        nc.sync.dma_start(out=Wo_t[:], in_=Wo_ext[:])
        rjl = cpool.tile([128, T], dt.bfloat16)
        nc.sync.dma_start(out=rjl[:], in_=rjl_ext[:])
        wg = cpool.tile([128, T], dt.bfloat16)
        nc.sync.dma_start(out=wg[:], in_=wg_ext[:])
        rix = cpool.tile([128, S // 16], dt.int16)
        nc.sync.dma_start(out=rix[:], in_=rix_ext[:])
        lix = cpool.tile([128, S // 16], dt.int16)
        nc.sync.dma_start(out=lix[:], in_=lix_ext[:])

        acc = accp.tile([128, NW, 64], dt.float32)
        nc.vector.memset(acc[:], 0.0)

        # ---- build rhs table [NPC,128] (cols 0:64 = input@Wi+bi, bf16) ----
        for w in range(NW):
            n0 = w * W
            n1 = min(NPC, n0 + W)
            m = n1 - n0
            aT = tabp.tile([65, 128], dt.bfloat16, tag="aT")
            nc.sync.dma_start(out=aT[:, :m], in_=inT_ext[:, n0:n1])
            ps = psA.tile([128, 64], dt.float32, tag="tps")
            nc.tensor.matmul(out=ps[:m, :], lhsT=aT[:, :m], rhs=WiB[:], start=True, stop=True)
            sb = tabp.tile([128, 64], dt.bfloat16, tag="tsb")
            nc.vector.tensor_copy(out=sb[:m, :], in_=ps[:m, :])
            nc.sync.dma_start(out=rtab[n0:n1, 0:64], in_=sb[:m, :])
        # ---- build lhs table [N_OT,128] ----
        NWO = (N_OT + W - 1) // W
        for w in range(NWO):
            n0 = w * W
            n1 = min(N_OT, n0 + W)
            m = n1 - n0
            aT = tabp.tile([64, 128], dt.bfloat16, tag="bT")
            nc.sync.dma_start(out=aT[:, :m], in_=otT_ext[:, n0:n1])
            ps = psA.tile([128, 64], dt.float32, tag="tps")
            nc.tensor.matmul(out=ps[:m, :], lhsT=aT[:, :m], rhs=Wo_t[:], start=True, stop=True)
            sb = tabp.tile([128, 64], dt.bfloat16, tag="tsb")
            nc.vector.tensor_copy(out=sb[:m, :], in_=ps[:m, :])
            nc.sync.dma_start(out=ltab[n0:n1, 0:64], in_=sb[:m, :])

        # ---- main loop: gather blocks, per-tile message + window scatter ----
        # track PSUM window accumulation: flush when window changes
        cur = {"w": None, "seg": None, "ps": None, "n0": None}

        def flush():
            if cur["ps"] is not None:
                wv = cur["w"]
                nc.vector.tensor_tensor(out=acc[:, wv, :], in0=acc[:, wv, :],
                                        in1=cur["ps"][:], op=mybir.AluOpType.add)
                cur["ps"] = None

        t_global = 0
        for b in range(NBLK):
            segb = 0 if (b * BLK) < TA_tok else 1
            tab_ap = ltab[0:SEG_SPLIT, :] if segb == 0 else ltab[SEG_SPLIT:N_OT, :]
            gl = gp.tile([128, TPB, 128], dt.bfloat16, tag="gl")
            nc.gpsimd.dma_gather(gl[:], tab_ap, lix[:, b * (BLK // 16):(b + 1) * (BLK // 16)],
                                 BLK, BLK, 128, queue_num=b % 4)
            gr = gp.tile([128, TPB, 128], dt.bfloat16, tag="gr")
            nc.gpsimd.dma_gather(gr[:], rtab[:], rix[:, b * (BLK // 16):(b + 1) * (BLK // 16)],
                                 BLK, BLK, 128, queue_num=(b + 2) % 4)
            msum = wk.tile([128, TPB, 64], dt.bfloat16, tag="msum")
            nc.vector.tensor_tensor(out=msum[:], in0=gl[:, :, 0:64], in1=gr[:, :, 0:64],
                                    op=mybir.AluOpType.add)
            t0 = t_global
            oh_all = ohp.tile([128, TPB, 128], dt.bfloat16, tag="ohall")
            nc.vector.tensor_tensor(
                out=oh_all[:], in0=iota[:, None, :].to_broadcast([128, TPB, 128]),
                in1=rjl[:, t0:t0 + TPB, None].to_broadcast([128, TPB, 128]),
                op=mybir.AluOpType.is_equal)
            ohs = [oh_all[:, i, :] for i in range(TPB)]
            wwe_all = wk.tile([128, TPB, 64], dt.bfloat16, tag="wweall")
            nc.vector.tensor_tensor(
                out=wwe_all[:], in0=wg[:, t0:t0 + TPB, None].to_broadcast([128, TPB, 64]),
                in1=WeR[:, None, :].to_broadcast([128, TPB, 64]),
                op=mybir.AluOpType.mult)
            m2b = wk.tile([128, TPB, 64], dt.bfloat16, tag="m2b")
            nc.vector.tensor_tensor(out=m2b[:], in0=msum[:], in1=wwe_all[:],
                                    op=mybir.AluOpType.add)
            mrb = wk.tile([128, TPB, 64], dt.bfloat16, tag="mrb")
            nc.scalar.activation(out=mrb[:], in_=m2b[:],
                                 func=mybir.ActivationFunctionType.Lrelu, alpha=0.01)
            for i in range(TPB):
                t = t_global
                t_global += 1
                sg, wv = sched[t]
                if wv < 0:
                    continue
                if cur["w"] != wv or cur["seg"] != sg:
                    flush()
                    cur["w"], cur["seg"] = wv, sg
                    cur["ps"] = psW.tile([128, 64], dt.float32, tag="psw", name=f"psw{t}")
                    cur["first"] = True
                nc.tensor.matmul(out=cur["ps"][:], lhsT=ohs[i], rhs=mrb[:, i, :],
                                 start=cur["first"], stop=False)
                cur["first"] = False
        flush()

        # ---- final: y = S@M1 + counts x v1 + 1 x bout + input@W2 ----
        M1t = cpool.tile([64, 64], dt.bfloat16)
        nc.sync.dma_start(out=M1t[:], in_=M1_ext[:])
        W2t = cpool.tile([64, 64], dt.bfloat16)
        nc.sync.dma_start(out=W2t[:], in_=W2_ext[:])
        vbt = cpool.tile([2, 64], dt.bfloat16)
        nc.sync.dma_start(out=vbt[:], in_=vb_ext[:])
        cntr = cpool.tile([2, NPC], dt.bfloat16)
        nc.sync.dma_start(out=cntr[:], in_=cnts_ext[:])
        ident = cpool.tile([128, 128], dt.bfloat16)
        from concourse.masks import make_identity
        make_identity(nc, ident[:])

        for w in range(NW):
            n0 = w * W
            n1 = min(NPC, n0 + W)
            m = n1 - n0
            # S_w bf16 and its transpose
            swb = wk.tile([128, 64], dt.bfloat16, tag="swb")
            nc.vector.tensor_copy(out=swb[:], in_=acc[:, w, :])
            tps = psA.tile([128, 128], dt.bfloat16, tag="fps")
            nc.tensor.transpose(out=tps[0:64, :], in_=swb[:], identity=ident[:])
            swT = wk.tile([64, 128], dt.bfloat16, tag="swT")
            nc.vector.tensor_copy(out=swT[:], in_=tps[0:64, :])
            inw = wk.tile([65, 128], dt.bfloat16, tag="inw")
            nc.sync.dma_start(out=inw[:, :m], in_=inT_ext[:, n0:n1])
            ops = psA.tile([128, 64], dt.float32, tag="ops")
            nc.tensor.matmul(out=ops[:m, :], lhsT=swT[:, :m], rhs=M1t[:], start=True, stop=False)
            nc.tensor.matmul(out=ops[:m, :], lhsT=inw[0:64, :m], rhs=W2t[:], start=False, stop=False)
            nc.tensor.matmul(out=ops[:m, :], lhsT=cntr[:, n0:n1], rhs=vbt[:], start=False, stop=True)
            ob = wk.tile([128, 64], dt.float32, tag="ob")
            nc.vector.tensor_copy(out=ob[:m, :], in_=ops[:m, :])
            nc.sync.dma_start(out=y_ext[n0:n1, :], in_=ob[:m, :])

    nc.compile()

    # ---------------- host-side in_maps ----------------
    W1 = Wout[:64]; W2 = Wout[64:]
    M1 = (Wf @ W1).astype(np.float32)
    v1 = (bf @ W1).astype(np.float32)
    vb = np.stack([v1, bout]).astype(bf16)
    iota_np = np.tile(np.arange(128, dtype=np.float32)[None, :], (128, 1)).astype(bf16)
    WeR_np = np.tile(We[None, :], (128, 1)).astype(bf16)
    WiB_np = np.concatenate([Wi, bi[None, :]], 0).astype(bf16)

    in_maps = []
    for c in range(NC):
        sl = input[c * NPC:(c + 1) * NPC]
        inT = np.concatenate([sl.T, np.ones((1, NPC), np.float32)], 0).astype(bf16)
        in_maps.append({
            "inT": np.ascontiguousarray(inT),
            "otT": np.ascontiguousarray(other.T).astype(bf16),
            "WiB": WiB_np, "Wo_": Wo.astype(bf16),
            "M1_": M1.astype(bf16), "W2_": W2.astype(bf16), "vb_": vb,
            "cnts": np.stack([counts[c], np.ones(NPC, np.float32)]).astype(bf16),
            "WeR": WeR_np, "iot": iota_np,
            "rix": _wrap16(rhs_idx[c]), "lix": _wrap16(lhs_idx[c]),
            "rjl": grid_pt(rjl_grid[c], bf16),
            "wg": grid_pt(w_grid[c], bf16),
        })

    import os
    res = run_bass_kernel_spmd(nc, in_maps, list(range(NC)),
                               trace=bool(os.environ.get("KTRACE")))
    if os.environ.get("KTRACE") and res.exec_time_ns:
        print(f"HW exec time: {res.exec_time_ns} ns")
    out = np.concatenate([res.results[c]["y"] for c in range(NC)], 0)
    return out.astype(np.float32)



# revision 12
# speedup vs baseline: 2.2100x; 1.1510x over previous
"""BipartiteGConv Trainium2 kernel (8 NeuronCores, edge/node sharding).

Math (see reference):
  rhs = input @ Wi + bi            [N_IN, D]
  lhs = other @ Wo                 [N_OT, D]
  msg = lrelu(rhs[rj] + lhs[lj] + w*We) per edge
  S   = segment_sum(msg, rj)       [N_IN, D]
  out = concat([S @ Wf + bf, input]) @ Wout + bout
      = S @ (Wf@W1) + counts x (bf@W1) + input @ W2 + bout   (W1=Wout[:D], W2=Wout[D:])

Sharding: nodes (rj ranges of 12500) across 8 cores; each core owns all
edges targeting its range.  Slots ordered by (lj-segment, rj-window of 128
nodes), padded per (window, seg) to uniform tile counts across cores
(SPMD).  Gathers via dma_gather (bf16-padded 256B rows); segment-sum via
per-tile onehot matmul accumulated in PSUM per window.
"""
import sys
sys.path.insert(0, "/opt/trn_rl_repo")
import numpy as np
import ml_dtypes

N_IN, N_OT, E, D = 100000, 50000, 1000000, 64
NC = 8
NPC = N_IN // NC            # nodes per core
W = 128                     # window size (nodes)
NW = (NPC + W - 1) // W     # windows per core
SEG_SPLIT = 32768           # lhs table A/B split (int16 idx limit)
BLK = 1024                  # gather tokens per instruction
PADV = 999.0                # rj_local value for pad slots (onehot -> 0)


def _wrap16(a):
    # token i -> [i % 16, i // 16], replicated to 128 partitions
    n = a.shape[0]
    assert n % 16 == 0
    return np.tile(a.reshape(n // 16, 16).T, (8, 1)).copy()


def kernel(input, other, rj, lj, weights, Wi, bi, Wo, We, Wf, bf, Wout, bout):
    import concourse.bass as bass
    import concourse.bacc as bacc
    import concourse.mybir as mybir
    import concourse.tile as tile
    from concourse.bass_utils import run_bass_kernel_spmd
    from contextlib import ExitStack

    input = np.asarray(input, np.float32)
    other = np.asarray(other, np.float32)
    rj = np.asarray(rj).astype(np.int64)
    lj = np.asarray(lj).astype(np.int64)
    weights = np.asarray(weights, np.float32).reshape(-1)
    Wi = np.asarray(Wi, np.float32); bi = np.asarray(bi, np.float32)
    Wo = np.asarray(Wo, np.float32); We = np.asarray(We, np.float32).reshape(-1)
    Wf = np.asarray(Wf, np.float32); bf = np.asarray(bf, np.float32)
    Wout = np.asarray(Wout, np.float32); bout = np.asarray(bout, np.float32)

    bf16 = ml_dtypes.bfloat16

    # ---------------- host index prep (per core) ----------------
    core_of = rj // NPC
    order0 = np.argsort(core_of, kind="stable")
    # per (core, seg, window) edge lists
    tiles_per = np.zeros((NC, 2, NW), np.int64)
    core_data = []
    for c in range(NC):
        sel = order0[np.searchsorted(core_of, c, side="left", sorter=order0):
                     np.searchsorted(core_of, c, side="right", sorter=order0)]
        sel = order0[core_of[order0] == c] if False else sel
        rjl_all = rj[sel] - c * NPC
        win = rjl_all // W
        seg = (lj[sel] >= SEG_SPLIT).astype(np.int64)
        key = seg * NW + win
        o2 = np.argsort(key, kind="stable")
        core_data.append((sel[o2], (rjl_all % W)[o2], key[o2]))
        cnt = np.bincount(key[o2], minlength=2 * NW).reshape(2, NW)
        tiles_per[c] = (cnt + 127) // 128
    TW = tiles_per.max(axis=0)          # uniform tiles per (seg, window)
    # segment A tile count rounded so segment boundary is BLK-aligned
    TA = int(TW[0].sum()); TB = int(TW[1].sum())
    padA = (-TA) % (BLK // 128)
    padB = (-(TA + padA + TB)) % (BLK // 128)
    # tile schedule: list of (seg, window) per tile, with pad tiles (seg, -1)
    sched = []
    for w in range(NW):
        sched += [(0, w)] * int(TW[0, w])
    sched += [(0, -1)] * padA
    for w in range(NW):
        sched += [(1, w)] * int(TW[1, w])
    sched += [(1, -1)] * padB
    T = len(sched)
    S = T * 128
    TA_tok = (TA + padA) * 128          # segment A token count (BLK-aligned)

    rhs_idx = np.zeros((NC, S), np.int16)
    lhs_idx = np.zeros((NC, S), np.int16)
    rjl_grid = np.full((NC, S), PADV, np.float32)
    w_grid = np.zeros((NC, S), np.float32)
    counts = np.zeros((NC, NPC), np.float32)
    for c in range(NC):
        sel, rjl_loc, key = core_data[c]
        counts[c] = np.bincount(rj[sel] - c * NPC, minlength=NPC)
        # slot positions: walk schedule, fill each (seg,window) group
        pos = 0
        ptr = 0  # pointer into sel
        for (sg, w) in sched:
            if w < 0:
                pos += 128
                continue
            k = sg * NW + w
            # edges for this (seg, window) in this core
            lo = np.searchsorted(key, k, side="left")
            hi = np.searchsorted(key, k, side="right")
            ne = hi - lo
            ntile = int(TW[sg, w])
            # how many already consumed for this key from earlier tiles
            # (groups are contiguous; fill greedily tile by tile)
            take = min(128, ne - (ptr if False else 0))
            # simpler: fill the whole group at its first tile encounter
            if lo != hi:
                e0 = np.arange(lo, hi)
                p0 = pos  # this is the first tile of the group only if we track it
            pos += 128
        # vectorized fill instead (group-contiguous):
        pos_of_group = {}
        p = 0
        for (sg, w) in sched:
            if w >= 0 and (sg, w) not in pos_of_group:
                pos_of_group[(sg, w)] = p
            p += 128
        for sg in range(2):
            for w in range(NW):
                k = sg * NW + w
                lo = np.searchsorted(key, k, side="left")
                hi = np.searchsorted(key, k, side="right")
                if lo == hi:
                    continue
                base = pos_of_group[(sg, w)]
                idxs = np.arange(base, base + (hi - lo))
                ee = sel[lo:hi]
                rhs_idx[c, idxs] = (rj[ee] - c * NPC).astype(np.int16)
                lv = lj[ee] - sg * SEG_SPLIT
                lhs_idx[c, idxs] = lv.astype(np.int16)
                rjl_grid[c, idxs] = rjl_loc[lo:hi].astype(np.float32)
                w_grid[c, idxs] = weights[ee]

    # slot s maps to (p, t) = (s % 128, s // 128) [dma_gather token layout]
    def grid_pt(a, dt):
        return np.ascontiguousarray(a.reshape(T, 128).T).astype(dt)

    # ---------------- build bass kernel ----------------
    dt = mybir.dt
    nc = bacc.Bacc("TRN2", target_bir_lowering=False, debug=False,
                   num_devices=NC, num_swdge_queues=4)

    inT_ext = nc.dram_tensor("inT", [65, NPC], dt.bfloat16, kind="ExternalInput").ap()
    otT_ext = nc.dram_tensor("otT", [64, N_OT], dt.bfloat16, kind="ExternalInput").ap()
    WiB_ext = nc.dram_tensor("WiB", [65, 64], dt.bfloat16, kind="ExternalInput").ap()
    Wo_ext = nc.dram_tensor("Wo_", [64, 64], dt.bfloat16, kind="ExternalInput").ap()
    M1_ext = nc.dram_tensor("M1_", [64, 64], dt.bfloat16, kind="ExternalInput").ap()
    W2_ext = nc.dram_tensor("W2_", [64, 64], dt.bfloat16, kind="ExternalInput").ap()
    vb_ext = nc.dram_tensor("vb_", [2, 64], dt.bfloat16, kind="ExternalInput").ap()
    cnts_ext = nc.dram_tensor("cnts", [2, NPC], dt.bfloat16, kind="ExternalInput").ap()
    WeR_ext = nc.dram_tensor("WeR", [128, 64], dt.bfloat16, kind="ExternalInput").ap()
    iota_ext = nc.dram_tensor("iot", [128, 128], dt.bfloat16, kind="ExternalInput").ap()
    rix_ext = nc.dram_tensor("rix", [128, S // 16], dt.int16, kind="ExternalInput").ap()
    lix_ext = nc.dram_tensor("lix", [128, S // 16], dt.int16, kind="ExternalInput").ap()
    rjl_ext = nc.dram_tensor("rjl", [128, T], dt.bfloat16, kind="ExternalInput").ap()
    wg_ext = nc.dram_tensor("wg", [128, T], dt.bfloat16, kind="ExternalInput").ap()
    y_ext = nc.dram_tensor("y", [NPC, 64], dt.float32, kind="ExternalOutput").ap()

    rtab = nc.dram_tensor("rtab", [NPC, 128], dt.bfloat16).ap()
    ltabA = nc.dram_tensor("ltabA", [SEG_SPLIT, 128], dt.bfloat16).ap()
    ltabB = nc.dram_tensor("ltabB", [N_OT - SEG_SPLIT, 128], dt.bfloat16).ap()

    NBLK = S // BLK
    TPB = BLK // 128  # tiles per block = 8

    with tile.TileContext(nc) as tc, ExitStack() as ctx:
        cpool = ctx.enter_context(tc.tile_pool(name="const", bufs=1))
        tabp = ctx.enter_context(tc.tile_pool(name="tab", bufs=3))
        gp = ctx.enter_context(tc.tile_pool(name="gath", bufs=6))
        wk = ctx.enter_context(tc.tile_pool(name="work", bufs=4))
        ohp = ctx.enter_context(tc.tile_pool(name="ohp", bufs=2))
        psA = ctx.enter_context(tc.tile_pool(name="psA", bufs=2, space="PSUM"))
        psW = ctx.enter_context(tc.tile_pool(name="psW", bufs=2, space="PSUM"))
        accp = ctx.enter_context(tc.tile_pool(name="acc", bufs=1))

        iota = cpool.tile([128, 128], dt.bfloat16)
        nc.sync.dma_start(out=iota[:], in_=iota_ext[:])
        WeR = cpool.tile([128, 64], dt.bfloat16)
        nc.sync.dma_start(out=WeR[:], in_=WeR_ext[:])
        WiB = cpool.tile([65, 64], dt.bfloat16)
        nc.sync.dma_start(out=WiB[:], in_=WiB_ext[:])
        Wo_t = cpool.tile([64, 64], dt.bfloat16)
        nc.sync.dma_start(out=Wo_t[:], in_=Wo_ext[:])
        rjl = cpool.tile([128, T], dt.bfloat16)
        nc.sync.dma_start(out=rjl[:], in_=rjl_ext[:])
        wg = cpool.tile([128, T], dt.bfloat16)
        nc.sync.dma_start(out=wg[:], in_=wg_ext[:])
        rix = cpool.tile([128, S // 16], dt.int16)
        nc.sync.dma_start(out=rix[:], in_=rix_ext[:])
        lix = cpool.tile([128, S // 16], dt.int16)
        nc.sync.dma_start(out=lix[:], in_=lix_ext[:])

        acc = accp.tile([128, NW, 64], dt.float32)
        nc.vector.memset(acc[:], 0.0)

        # ---- chunked table builds: big input DMAs, 8 matmuls per PSUM
        # bank, one DVE convert per 8 row-groups, per-group out DMAs ----
        def build_tab(dst, src_ext, n_rows, wrows, wmat, roff=0, CH=2048):
            done = 0
            while done < n_rows:
                todo = min(CH, n_rows - done)
                srct = tabp.tile([65, CH], dt.bfloat16, tag="bsrc")
                nc.sync.dma_start(out=srct[:wrows, :todo],
                                  in_=src_ext[:, roff + done:roff + done + todo])
                ngr = (todo + 127) // 128
                if todo < ngr * 128:
                    nc.vector.memset(srct[:wrows, todo:ngr * 128], 0.0)
                for k0 in range(0, ngr, 8):
                    kn = min(8, ngr - k0)
                    ps = psA.tile([128, 512], dt.float32, tag="bps")
                    for k in range(kn):
                        r0 = (k0 + k) * 128
                        nc.tensor.matmul(out=ps[:, k * 64:(k + 1) * 64],
                                         lhsT=srct[:wrows, r0:r0 + 128],
                                         rhs=wmat[:], start=True, stop=True)
                    sb = tabp.tile([128, 8, 64], dt.bfloat16, tag="bsb")
                    nc.vector.tensor_copy(
                        out=sb[:, :kn, :].rearrange("p k d -> p (k d)"),
                        in_=ps[:, :kn * 64])
                    for k in range(kn):
                        rr = done + (k0 + k) * 128
                        mm = min(128, n_rows - rr)
                        nc.sync.dma_start(out=dst[rr:rr + mm, 0:64],
                                          in_=sb[:mm, k, :])
                done += todo

        build_tab(rtab, inT_ext, NPC, 65, WiB)
        build_tab(ltabA, otT_ext, SEG_SPLIT, 64, Wo_t)
        build_tab(ltabB, otT_ext, N_OT - SEG_SPLIT, 64, Wo_t, roff=SEG_SPLIT)

        # ---- main loop: gather blocks, per-tile message + window scatter ----
        # track PSUM window accumulation: flush when window changes
        cur = {"w": None, "seg": None, "ps": None, "n0": None}

        def flush():
            if cur["ps"] is not None:
                wv = cur["w"]
                nc.vector.tensor_tensor(out=acc[:, wv, :], in0=acc[:, wv, :],
                                        in1=cur["ps"][:], op=mybir.AluOpType.add)
                cur["ps"] = None

        t_global = 0
        for b in range(NBLK):
            segb = 0 if (b * BLK) < TA_tok else 1
            tab_ap = ltabA[:, :] if segb == 0 else ltabB[:, :]
            gl = gp.tile([128, TPB, 128], dt.bfloat16, tag="gl")
            nc.gpsimd.dma_gather(gl[:], tab_ap, lix[:, b * (BLK // 16):(b + 1) * (BLK // 16)],
                                 BLK, BLK, 128, queue_num=b % 4)
            gr = gp.tile([128, TPB, 128], dt.bfloat16, tag="gr")
            nc.gpsimd.dma_gather(gr[:], rtab[:], rix[:, b * (BLK // 16):(b + 1) * (BLK // 16)],
                                 BLK, BLK, 128, queue_num=(b + 2) % 4)
            msum = wk.tile([128, TPB, 64], dt.bfloat16, tag="msum")
            nc.vector.tensor_tensor(out=msum[:], in0=gl[:, :, 0:64], in1=gr[:, :, 0:64],
                                    op=mybir.AluOpType.add)
            t0 = t_global
            oh_all = ohp.tile([128, TPB, 128], dt.bfloat16, tag="ohall")
            nc.vector.tensor_tensor(
                out=oh_all[:], in0=iota[:, None, :].to_broadcast([128, TPB, 128]),
                in1=rjl[:, t0:t0 + TPB, None].to_broadcast([128, TPB, 128]),
                op=mybir.AluOpType.is_equal)
            ohs = [oh_all[:, i, :] for i in range(TPB)]
            wwe_all = wk.tile([128, TPB, 64], dt.bfloat16, tag="wweall")
            nc.vector.tensor_tensor(
                out=wwe_all[:], in0=wg[:, t0:t0 + TPB, None].to_broadcast([128, TPB, 64]),
                in1=WeR[:, None, :].to_broadcast([128, TPB, 64]),
                op=mybir.AluOpType.mult)
            m2b = wk.tile([128, TPB, 64], dt.bfloat16, tag="m2b")
            nc.vector.tensor_tensor(out=m2b[:], in0=msum[:], in1=wwe_all[:],
                                    op=mybir.AluOpType.add)
            mrb = wk.tile([128, TPB, 64], dt.bfloat16, tag="mrb")
            nc.scalar.activation(out=mrb[:], in_=m2b[:],
                                 func=mybir.ActivationFunctionType.Lrelu, alpha=0.01)
            for i in range(TPB):
                t = t_global
                t_global += 1
                sg, wv = sched[t]
                if wv < 0:
                    continue
                if cur["w"] != wv or cur["seg"] != sg:
                    flush()
                    cur["w"], cur["seg"] = wv, sg
                    cur["ps"] = psW.tile([128, 64], dt.float32, tag="psw", name=f"psw{t}")
                    cur["first"] = True
                nc.tensor.matmul(out=cur["ps"][:], lhsT=ohs[i], rhs=mrb[:, i, :],
                                 start=cur["first"], stop=False)
                cur["first"] = False
        flush()

        # ---- final: y = S@M1 + counts x v1 + 1 x bout + input@W2 ----
        M1t = cpool.tile([64, 64], dt.bfloat16)
        nc.sync.dma_start(out=M1t[:], in_=M1_ext[:])
        W2t = cpool.tile([64, 64], dt.bfloat16)
        nc.sync.dma_start(out=W2t[:], in_=W2_ext[:])
        vbt = cpool.tile([2, 64], dt.bfloat16)
        nc.sync.dma_start(out=vbt[:], in_=vb_ext[:])
        cntr = cpool.tile([2, NPC], dt.bfloat16)
        nc.sync.dma_start(out=cntr[:], in_=cnts_ext[:])
        ident = cpool.tile([128, 128], dt.bfloat16)
        from concourse.masks import make_identity
        make_identity(nc, ident[:])

        for w in range(NW):
            n0 = w * W
            n1 = min(NPC, n0 + W)
            m = n1 - n0
            # S_w bf16 and its transpose
            swb = wk.tile([128, 64], dt.bfloat16, tag="swb")
            nc.vector.tensor_copy(out=swb[:], in_=acc[:, w, :])
            tps = psA.tile([128, 128], dt.bfloat16, tag="fps")
            nc.tensor.transpose(out=tps[0:64, :], in_=swb[:], identity=ident[:])
            swT = wk.tile([64, 128], dt.bfloat16, tag="swT")
            nc.vector.tensor_copy(out=swT[:], in_=tps[0:64, :])
            inw = wk.tile([65, 128], dt.bfloat16, tag="inw")
            nc.sync.dma_start(out=inw[:, :m], in_=inT_ext[:, n0:n1])
            ops = psA.tile([128, 64], dt.float32, tag="ops")
            nc.tensor.matmul(out=ops[:m, :], lhsT=swT[:, :m], rhs=M1t[:], start=True, stop=False)
            nc.tensor.matmul(out=ops[:m, :], lhsT=inw[0:64, :m], rhs=W2t[:], start=False, stop=False)
            nc.tensor.matmul(out=ops[:m, :], lhsT=cntr[:, n0:n1], rhs=vbt[:], start=False, stop=True)
            ob = wk.tile([128, 64], dt.float32, tag="ob")
            nc.vector.tensor_copy(out=ob[:m, :], in_=ops[:m, :])
            nc.sync.dma_start(out=y_ext[n0:n1, :], in_=ob[:m, :])

    nc.compile()

    # ---------------- host-side in_maps ----------------
    W1 = Wout[:64]; W2 = Wout[64:]
    M1 = (Wf @ W1).astype(np.float32)
    v1 = (bf @ W1).astype(np.float32)
    vb = np.stack([v1, bout]).astype(bf16)
    iota_np = np.tile(np.arange(128, dtype=np.float32)[None, :], (128, 1)).astype(bf16)
    WeR_np = np.tile(We[None, :], (128, 1)).astype(bf16)
    WiB_np = np.concatenate([Wi, bi[None, :]], 0).astype(bf16)

    in_maps = []
    for c in range(NC):
        sl = input[c * NPC:(c + 1) * NPC]
        inT = np.concatenate([sl.T, np.ones((1, NPC), np.float32)], 0).astype(bf16)
        in_maps.append({
            "inT": np.ascontiguousarray(inT),
            "otT": np.ascontiguousarray(other.T).astype(bf16),
            "WiB": WiB_np, "Wo_": Wo.astype(bf16),
            "M1_": M1.astype(bf16), "W2_": W2.astype(bf16), "vb_": vb,
            "cnts": np.stack([counts[c], np.ones(NPC, np.float32)]).astype(bf16),
            "WeR": WeR_np, "iot": iota_np,
            "rix": _wrap16(rhs_idx[c]), "lix": _wrap16(lhs_idx[c]),
            "rjl": grid_pt(rjl_grid[c], bf16),
            "wg": grid_pt(w_grid[c], bf16),
        })

    import os
    res = run_bass_kernel_spmd(nc, in_maps, list(range(NC)),
                               trace=bool(os.environ.get("KTRACE")))
    if os.environ.get("KTRACE") and res.exec_time_ns:
        print(f"HW exec time: {res.exec_time_ns} ns")
    out = np.concatenate([res.results[c]["y"] for c in range(NC)], 0)
    return out.astype(np.float32)

